# revision 1
# baseline (speedup 1.0000x reference)
"""Trainium2 Bass kernel for DifferentiableSupergraphDynamics.

Computation:
    edge_w = where(learn_mask, tanh(theta), sign*conf) * delay      [E]
    msgs   = x[:, src] * edge_w                                     [B, E]
    agg    = scatter_add(msgs -> dst)                               [B, N]
    rate   = base_rate * exp(rate_log_scale)                        [N]
    drive  = tanh(agg + bias)
    x_next = clip(x + DT * rate * (drive*cap - x), 0, cap)

Sharding: destination nodes are dealt round-robin (by total in-degree
rank) across the 8 cores; every edge lives on its destination's core, so
no cross-core collective is needed.

Per-core edge phase: edges are split into (up to) 4 "structures" by
source-node range (32768 rows each, so dma_gather's int16 indices can
address the x table). Each structure is a padded CSR over the core's
nodes sorted by that structure's in-degree: node groups of 128
partitions padded to the group max degree D. The x-row gather for all of
a structure's slots is done with the vectorized SWDGE dma_gather ucode
(one 64B descriptor per slot, round-robin over the 4 SWDGE queues), the
weighted per-node reduction is a strided Vector-engine tensor_reduce,
and the 4 per-structure partial aggregates are merged into structure-0's
node order with unique-index dma_scatter_add through HBM.
"""

import numpy as np

import concourse.bass as bass
import concourse.bacc as bacc
import concourse.mybir as mybir
import concourse.tile as tile
from concourse.bass_utils import run_bass_kernel_spmd

P = 128
NCORES = 8
DT = 0.1
SRC_CHUNK = 32768          # dma_gather int16 index reach
GATHER_CALL = 8192         # SWDGE ring capacity per call
XBF16 = False              # f32 x rows (64B descriptors); bf16 trips clip-boundary rel err
ROWE = 64                  # x-table row stride: 256B (dma_gather req)
XROW = 128 if XBF16 else 64

F32 = mybir.dt.float32
I16 = mybir.dt.int16
I8 = mybir.dt.int8


def _wrap_idx(flat, call):
    """Lay a flat int16 index list out in the SWDGE wrapped layout:
    per call of `call` indices, index j at [j%16, j//16]; 32-partition
    replicated (descriptor-gen runs on two Q7 cores)."""
    n = len(flat)
    ncall = (n + call - 1) // call
    pad = ncall * call - n
    if pad:
        flat = np.concatenate([flat, np.zeros(pad, flat.dtype)])
    cols = np.concatenate(
        [flat[k * call:(k + 1) * call].reshape(call // 16, 16).T
         for k in range(ncall)], axis=1)          # [16, ncall*call/16]
    return np.concatenate([cols] * 8, axis=0)     # [128, ...]


# ---------------------------------------------------------------------------
# Host-side data preparation
# ---------------------------------------------------------------------------

def _prep(x, theta, bias, ratelog, baserate, cap, sign, conf, delay, src, dst,
          mask, n_cores):
    B, N = x.shape
    E = src.shape[0]

    src = np.asarray(src).astype(np.int64)
    dst = np.asarray(dst).astype(np.int64)
    theta = np.asarray(theta, dtype=np.float32)
    sign = np.asarray(sign, dtype=np.float32)
    conf = np.asarray(conf, dtype=np.float32)
    delay = np.asarray(delay, dtype=np.float32)
    mask8 = np.asarray(mask).astype(np.int8)

    deg = np.bincount(dst, minlength=N)
    order = np.argsort(-deg, kind="stable")
    npc = (N + n_cores - 1) // n_cores
    G = (npc + P - 1) // P
    nper = G * P                                   # nodes per core (padded)

    rank_of = np.empty(N, dtype=np.int64)
    rank_of[order] = np.arange(N)
    core_of = rank_of % n_cores                    # node -> core
    pos_of = rank_of // n_cores                    # node -> position in core

    n_pad = ((N + ROWE - 1) // ROWE) * ROWE
    nq = (n_pad + SRC_CHUNK - 1) // SRC_CHUNK     # structures
    q_of = src // SRC_CHUNK                        # edge -> structure

    # per (core, structure) in-degree
    edge_core = core_of[dst]
    edge_pos = pos_of[dst]
    degq = np.zeros((n_cores, nper, nq), dtype=np.int64)
    np.add.at(degq, (edge_core, edge_pos, q_of), 1)

    # shared-over-cores placement per structure: within each core sort
    # positions by degq desc; group windows of 128; D = max over cores.
    D = np.zeros((nq, G), dtype=np.int64)
    ordq = np.zeros((n_cores, nq, nper), dtype=np.int64)   # row j -> position
    invq = np.zeros((n_cores, nq, nper), dtype=np.int64)   # position -> row j
    for q in range(nq):
        for c in range(n_cores):
            o = np.argsort(-degq[c, :, q], kind="stable")
            ordq[c, q] = o
            invq[c, q, o] = np.arange(nper)
            dm = degq[c, o, q].reshape(G, P).max(axis=1)
            D[q] = np.maximum(D[q], dm)
    D[0] = np.maximum(D[0], 1)       # canonical layout covers all nodes
    S = np.zeros((nq, G + 1), dtype=np.int64)
    S[:, 1:] = np.cumsum(D, axis=1)
    F = S[:, -1]                                   # cols per structure
    Gact = np.array([int((D[q] > 0).sum()) for q in range(nq)])

    # --- edge slot assignment ---
    # edge -> (core, structure, row=invq, occurrence within (node,structure))
    eord = np.lexsort((src, dst))                  # group by dst, then src q
    ec = edge_core[eord]
    ep = edge_pos[eord]
    eq = q_of[eord]
    # occurrence counter within (dst, q): edges sorted by (dst, q)
    key_change = np.ones(E, dtype=bool)
    key_change[1:] = (dst[eord][1:] != dst[eord][:-1]) | (eq[1:] != eq[:-1])
    run_id = np.cumsum(key_change) - 1
    run_starts = np.flatnonzero(key_change)
    occ = np.arange(E) - run_starts[run_id]

    row = invq[ec, eq, ep]                         # row index in structure
    g = row // P
    pp = row % P
    col = S[eq, g] + occ
    # slot linear index within (core, structure): i = pp + 128*col
    slot_i = pp + P * col

    # per (core, structure) arrays
    ins = []
    FT = int(F.sum())                              # total cols, all structures
    Scol = np.zeros(nq + 1, dtype=np.int64)
    Scol[1:] = np.cumsum(F)

    # params laid out [P, FT] per core (slot (q,p,col) -> [p, Scol[q]+col])
    par_shape = (n_cores, P, FT)
    thetaA = np.zeros(par_shape, np.float32)
    signA = np.zeros(par_shape, np.float32)
    confA = np.zeros(par_shape, np.float32)
    delayA = np.zeros(par_shape, np.float32)
    maskA = np.zeros(par_shape, np.int8)
    pidx = (ec, pp, Scol[eq] + col)
    thetaA[pidx] = theta[eord]
    signA[pidx] = sign[eord]
    confA[pidx] = conf[eord]
    delayA[pidx] = delay[eord]
    maskA[pidx] = mask8[eord]

    # gather index lists (wrapped) per core: concat over structures of
    # the per-structure slot-order index list (idx relative to q base)
    srcrel = (src[eord] - eq * SRC_CHUNK).astype(np.int16)
    gidx = []
    ncalls = np.zeros(nq, dtype=np.int64)
    for c in range(n_cores):
        parts = []
        for q in range(nq):
            tot = int(F[q]) * P
            a = np.zeros(tot, np.int16)
            selq = (ec == c) & (eq == q)
            a[slot_i[selq]] = srcrel[selq]
            parts.append(_wrap_idx(a, GATHER_CALL))
            ncalls[q] = (tot + GATHER_CALL - 1) // GATHER_CALL
        gidx.append(np.concatenate(parts, axis=1))
    gidx_cols = gidx[0].shape[1]

    # canonical placement = structure 0's; scatter index for structures
    # 1..nq-1: input position i = p + 128*g -> canonical slot p0*G + g0
    sidx = []
    for c in range(n_cores):
        parts = []
        for q in range(1, nq):
            node_pos = ordq[c, q]                  # row j -> position
            crow = invq[c, 0, node_pos]            # canonical row of node
            canon = (crow % P) * G + (crow // P)   # partition-major slot id
            a = np.zeros(nper, np.int16)
            jj = np.arange(nper)
            a[(jj % P) + P * (jj // P)] = canon.astype(np.int16)
            nact = int(Gact[q]) * P
            half = min((G // 2) * P, nact)
            parts.append(_wrap_idx(a[:half], half))
            if nact > half:
                parts.append(_wrap_idx(a[half:nact], nact - half))
        sidx.append(np.concatenate(parts, axis=1) if parts else
                    np.zeros((128, 16), np.int16))
    sidx_cols = sidx[0].shape[1]

    # node params in canonical placement [P, G]
    def node_arr(vals, fill):
        a = np.full((n_cores, P, G), fill, dtype=np.float32)
        for c in range(n_cores):
            node_pos = ordq[c, 0]                  # canonical row j -> pos
            rank = node_pos * n_cores + c          # position -> rank
            ok = rank < N
            nd = order[np.minimum(rank, N - 1)]
            v = np.where(ok, vals[nd], fill).astype(np.float32)
            a[c].reshape(-1)[(np.arange(nper) % P) * G +
                             (np.arange(nper) // P)] = np.where(
                                 ok, v, fill)
        return a

    biasA = node_arr(np.asarray(bias), 0.0)
    ratelogA = node_arr(np.asarray(ratelog), 0.0)
    baserateA = node_arr(np.asarray(baserate), 0.0)
    capA = node_arr(np.asarray(cap), 1.0)

    import ml_dtypes
    xdt = ml_dtypes.bfloat16 if XBF16 else np.float32
    xT4 = np.zeros((n_pad, XROW), xdt)
    xT4[:N, :B] = np.asarray(x, dtype=np.float32).T.astype(xdt)

    xTf = np.asarray(x, dtype=np.float32).T
    xownA = np.zeros((n_cores, P, G, B), np.float32)
    node_ids = np.zeros((n_cores, P, G), np.int64)
    for c in range(n_cores):
        node_pos = ordq[c, 0]
        rank = node_pos * n_cores + c
        ok = rank < N
        nd = np.where(ok, order[np.minimum(rank, N - 1)], -1)
        jj = np.arange(nper)
        pcol = (jj % P, jj // P)
        node_ids[c][pcol] = nd
        xownA[c][pcol[0], pcol[1], :] = np.where(
            ok[:, None], xTf[np.maximum(nd, 0), :], 0.0)

    for c in range(n_cores):
        ins.append({
            "xT4": xT4,
            "gidx": gidx[c],
            "sidx": sidx[c],
            "theta": thetaA[c],
            "sgn": signA[c],
            "conf": confA[c],
            "delay": delayA[c],
            "maskf": maskA[c],
            "bias": biasA[c],
            "ratelog": ratelogA[c],
            "baserate": baserateA[c],
            "cap": capA[c],
            "xown": xownA[c].reshape(P, G * B),
        })
    plan = dict(B=B, N=N, G=G, nq=nq, D=D, S=S, F=F, Scol=Scol, Gact=Gact,
                n_pad=n_pad, gidx_cols=gidx_cols, sidx_cols=sidx_cols,
                node_ids=node_ids)
    return ins, plan


def _assemble(results, plan):
    B, N, G = plan["B"], plan["N"], plan["G"]
    out = np.empty((B, N), dtype=np.float32)
    for ci, res in enumerate(results):
        o = res["out"].reshape(P, G, B)
        nid = plan["node_ids"][ci]
        ok = nid >= 0
        out[:, nid[ok]] = o[ok].T
    return out


# ---------------------------------------------------------------------------
# Device kernel
# ---------------------------------------------------------------------------

def _raw_dma_gather(g, out_ap, in_ap, idxs_ap, num_idxs, elem_size, elem_step,
                    queue_num):
    stride_bytes = elem_step * mybir.dt.size(in_ap.dtype)
    return g.add_instruction(
        mybir.InstDMAGatherAnt(
            name=g.bass.get_next_instruction_name(),
            ins=[*g.lower_ap_dma(in_ap, for_custom_bir_dma=True),
                 g.lower_ap(idxs_ap), g.lower_val_access(g.to_reg(num_idxs))],
            outs=[g.lower_ap(out_ap)],
            transpose=False, num_idxs=num_idxs, elem_size=elem_size,
            stride_bytes_256=stride_bytes // 256, gen_mode=0,
            single_packet=False, queue_num=queue_num,
            sbuf_tokens_per_rank=0, sbuf_free_dim_per_rank=0,
            sbuf_free_dim_pad_per_rank=0, sbuf_byte_offset=0))


def _equal_d_runs(D, g0, g1):
    runs = []
    a = g0
    while a < g1:
        b = a + 1
        while b < g1 and D[b] == D[a]:
            b += 1
        runs.append((a, b, int(D[a])))
        a = b
    return runs


def build(B, N, G, nq, D, S, F, Scol, n_pad, gidx_cols, sidx_cols,
          Gact=None, node_ids=None, enable_asserts=False, loop_r=None):
    if Gact is None:
        Gact = np.array([G] * nq)
    FT = int(Scol[-1])
    nc = bacc.Bacc("TRN2", target_bir_lowering=False, debug=False,
                   enable_asserts=enable_asserts, num_swdge_queues=4)

    XDT = mybir.dt.bfloat16 if XBF16 else F32
    xT4 = nc.dram_tensor("xT4", [n_pad, XROW], XDT, kind="ExternalInput")
    giD = nc.dram_tensor("gidx", [128, gidx_cols], I16, kind="ExternalInput")
    siD = nc.dram_tensor("sidx", [128, sidx_cols], I16, kind="ExternalInput")
    thD = nc.dram_tensor("theta", [P, FT], F32, kind="ExternalInput")
    sgD = nc.dram_tensor("sgn", [P, FT], F32, kind="ExternalInput")
    cfD = nc.dram_tensor("conf", [P, FT], F32, kind="ExternalInput")
    dlD = nc.dram_tensor("delay", [P, FT], F32, kind="ExternalInput")
    mkD = nc.dram_tensor("maskf", [P, FT], I8, kind="ExternalInput")
    biD = nc.dram_tensor("bias", [P, G], F32, kind="ExternalInput")
    rlD = nc.dram_tensor("ratelog", [P, G], F32, kind="ExternalInput")
    brD = nc.dram_tensor("baserate", [P, G], F32, kind="ExternalInput")
    cpD = nc.dram_tensor("cap", [P, G], F32, kind="ExternalInput")
    xoD = nc.dram_tensor("xown", [P, G * B], F32, kind="ExternalInput")
    outD = nc.dram_tensor("out", [P, G * B], F32, kind="ExternalOutput")
    # partial-agg merge buffers (zero-initialized by the runtime)
    pagg = [nc.dram_tensor(f"pagg{q}", [G * P, ROWE], F32,
                           kind="ExternalOutput") for q in range(1, nq)]

    Tanh = mybir.ActivationFunctionType.Tanh
    Exp = mybir.ActivationFunctionType.Exp

    qrr = [0]

    def next_q():
        qrr[0] = (qrr[0] + 1) % 4
        return qrr[0]

    import contextlib
    with tile.TileContext(nc) as tc:
        with (
            tc.tile_pool(name="persist", bufs=1) as ppool,
            tc.tile_pool(name="work", bufs=2) as wp,
            tc.tile_pool(name="msgs", bufs=2) as mp,
        ):
          with (tc.For_i(0, loop_r, 1) if loop_r else
                contextlib.nullcontext()):
            agg0 = ppool.tile([P, G * B], F32, tag="agg0")
            aggq_tiles = []

            qorder = list(range(1, nq)) + [0]
            gidx_bases = np.zeros(nq + 1, dtype=np.int64)
            sidx_bases = np.zeros(nq, dtype=np.int64)
            sb = 0
            for q in range(nq):
                tot = int(F[q]) * P
                ncall = (tot + GATHER_CALL - 1) // GATHER_CALL if tot else 0
                gidx_bases[q + 1] = gidx_bases[q] + ncall * (GATHER_CALL // 16)
                if q >= 1:
                    sidx_bases[q] = sb
                    nact = int(Gact[q]) * P
                    sb += (nact + 15) // 16
            for q in qorder:
                Fq = int(F[q])
                TOTq = Fq * P
                if TOTq == 0:
                    continue
                ncall = (TOTq + GATHER_CALL - 1) // GATHER_CALL
                icols = ncall * (GATHER_CALL // 16)
                gidx_t = wp.tile([128, icols], I16, tag="gidx")
                gb = int(gidx_bases[q])
                nc.sync.dma_start(
                    out=gidx_t[:],
                    in_=giD[:, gb:gb + icols])

                th = wp.tile([P, Fq], F32, tag="th")
                sg = wp.tile([P, Fq], F32, tag="sg")
                cf = wp.tile([P, Fq], F32, tag="cf")
                dl = wp.tile([P, Fq], F32, tag="dl")
                mk = wp.tile([P, Fq], I8, tag="mk")
                c0, c1 = int(Scol[q]), int(Scol[q + 1])
                nc.sync.dma_start(out=th[:], in_=thD[:, c0:c1])
                nc.sync.dma_start(out=sg[:], in_=sgD[:, c0:c1])
                nc.sync.dma_start(out=cf[:], in_=cfD[:, c0:c1])
                nc.sync.dma_start(out=dl[:], in_=dlD[:, c0:c1])
                nc.sync.dma_start(out=mk[:], in_=mkD[:, c0:c1])

                t = wp.tile([P, Fq], F32, tag="t")
                w = wp.tile([P, Fq], F32, tag="w")
                nc.scalar.activation(t[:], th[:], Tanh)
                nc.vector.tensor_mul(w[:], sg[:], cf[:])
                nc.vector.copy_predicated(w[:], mk[:], t[:])
                nc.vector.tensor_mul(w[:], w[:], dl[:])

                msgs = mp.tile([P, Fq * B], F32, tag="msgs")
                m3 = msgs[:].rearrange("p (s b) -> p s b", b=B)
                base = q * SRC_CHUNK
                in_ap = xT4[base:min(base + SRC_CHUNK, n_pad), :B]
                if XBF16:
                    msgsr = mp.tile([P, Fq * B], XDT, tag="msgsr")
                    gdst = msgsr[:].rearrange("p (s b) -> p s b", b=B)
                else:
                    gdst = m3
                for k in range(ncall):
                    i0 = k * GATHER_CALL
                    ni = min(GATHER_CALL, TOTq - i0)
                    _raw_dma_gather(
                        nc.gpsimd,
                        gdst[:, i0 // P:(i0 + ni) // P, :],
                        in_ap,
                        gidx_t[:, k * (GATHER_CALL // 16):
                               k * (GATHER_CALL // 16) + (ni + 15) // 16],
                        ni, B, XROW, next_q())

                wb = w[:].unsqueeze(-1).to_broadcast([P, Fq, B])
                if XBF16:
                    nc.vector.tensor_tensor(out=m3, in0=gdst, in1=wb,
                                            op=mybir.AluOpType.mult)
                else:
                    nc.vector.tensor_mul(m3, m3, wb)

                if q == 0:
                    aggt = agg0
                else:
                    aggt = wp.tile([P, G * B], F32, tag="aggq")
                    aggq_tiles.append(aggt)
                for (ga, gb2, d) in _equal_d_runs(D[q], 0, int(Gact[q])):
                    if d == 0:
                        continue
                    src_ap = (m3[:, int(S[q, ga]):int(S[q, gb2]), :]
                              .rearrange("p (n d) b -> p n b d", d=d))
                    dst_ap = aggt[:, ga * B:gb2 * B].rearrange(
                        "p (n b) -> p n b", b=B)
                    nc.vector.tensor_reduce(
                        dst_ap, src_ap, axis=mybir.AxisListType.X,
                        op=mybir.AluOpType.add)

                if q > 0:
                    # scatter active rows into canonical order through HBM
                    a3 = aggt[:].rearrange("p (g b) -> p g b", b=B)
                    nact = int(Gact[q]) * P
                    half = min((G // 2) * P, nact)
                    sbase = int(sidx_bases[q])
                    nc.gpsimd.dma_scatter_add(
                        pagg[q - 1][:, :B], a3[:, :half // P, :],
                        _slice_idx(wp, nc, siD, sbase, half),
                        half, half, B, elem_step=ROWE,
                        single_packet=False, queue_num=next_q())
                    if nact > half:
                        nc.gpsimd.dma_scatter_add(
                            pagg[q - 1][:, :B],
                            a3[:, half // P:nact // P, :],
                            _slice_idx(wp, nc, siD, sbase + half // 16,
                                       nact - half),
                            nact - half, nact - half, B, elem_step=ROWE,
                            single_packet=False, queue_num=next_q())

            # ---- merge + ODE epilogue ----
            rdb = []
            for q in range(1, nq):
                if int(Gact[q]) == 0:
                    continue
                rt = ppool.tile([P, G * B], F32, tag=f"rdb{q}")
                nc.sync.dma_start(
                    out=rt[:].rearrange("p (g b) -> p g b", b=B),
                    in_=pagg[q - 1][:, :B].rearrange(
                        "(p g) b -> p g b", p=P))
                rdb.append(rt)
            for rt in rdb:
                nc.vector.tensor_add(agg0[:], agg0[:], rt[:])

            bi = ppool.tile([P, G], F32, tag="bi")
            rl = ppool.tile([P, G], F32, tag="rl")
            br = ppool.tile([P, G], F32, tag="br")
            cp = ppool.tile([P, G], F32, tag="cp")
            xo = ppool.tile([P, G * B], F32, tag="xo")
            nc.sync.dma_start(out=bi[:], in_=biD[:, :])
            nc.sync.dma_start(out=rl[:], in_=rlD[:, :])
            nc.sync.dma_start(out=br[:], in_=brD[:, :])
            nc.sync.dma_start(out=cp[:], in_=cpD[:, :])
            nc.sync.dma_start(out=xo[:], in_=xoD[:, :])

            rate = ppool.tile([P, G], F32, tag="rate")
            nc.scalar.activation(rate[:], rl[:], Exp)
            nc.vector.tensor_mul(rate[:], rate[:], br[:])

            a3 = agg0[:].rearrange("p (g b) -> p g b", b=B)
            bib = bi[:].unsqueeze(-1).to_broadcast([P, G, B])
            cpb = cp[:].unsqueeze(-1).to_broadcast([P, G, B])
            rateb = rate[:].unsqueeze(-1).to_broadcast([P, G, B])

            dr = ppool.tile([P, G * B], F32, tag="dr")
            d3 = dr[:].rearrange("p (g b) -> p g b", b=B)
            nc.vector.tensor_add(d3, a3, bib)
            nc.scalar.activation(dr[:], dr[:], Tanh)
            nc.vector.tensor_mul(d3, d3, cpb)
            nc.vector.tensor_tensor(out=dr[:], in0=dr[:], in1=xo[:],
                                    op=mybir.AluOpType.subtract)
            nc.vector.tensor_mul(d3, d3, rateb)
            nc.vector.tensor_scalar_mul(dr[:], dr[:], float(DT))
            nc.vector.tensor_add(dr[:], dr[:], xo[:])
            nc.vector.tensor_scalar_max(dr[:], dr[:], 0.0)
            nc.vector.tensor_tensor(out=d3, in0=d3, in1=cpb,
                                    op=mybir.AluOpType.min)
            nc.sync.dma_start(out=outD[:, :], in_=dr[:])

    nc.compile()
    return nc


def _slice_idx(wp, nc, siD, col0, n):
    t = wp.tile([128, (n + 15) // 16], I16, tag="sidx")
    nc.sync.dma_start(out=t[:], in_=siD[:, col0:col0 + (n + 15) // 16])
    return t[:]


# ---------------------------------------------------------------------------
# Entry point
# ---------------------------------------------------------------------------

def kernel(x, theta_graph, node_bias, rate_log_scale, base_rate, capacity,
           sign_prior, conf_scale, delay_scale, src_index, dst_index,
           learn_mask):
    ins, plan = _prep(x, theta_graph, node_bias, rate_log_scale, base_rate,
                      capacity, sign_prior, conf_scale, delay_scale,
                      src_index, dst_index, learn_mask, NCORES)
    nc = build(plan["B"], plan["N"], plan["G"], plan["nq"], plan["D"],
               plan["S"], plan["F"], plan["Scol"], plan["n_pad"],
               plan["gidx_cols"], plan["sidx_cols"], Gact=plan["Gact"])
    res = run_bass_kernel_spmd(nc, ins, core_ids=list(range(NCORES)))
    return _assemble(res.results, plan)



# revision 2
# speedup vs baseline: 1.3876x; 1.3876x over previous
"""Trainium2 Bass kernel for DifferentiableSupergraphDynamics (CSC/scatter-add).

Computation:
    edge_w = where(learn_mask, tanh(theta), sign*conf) * delay      [E]
    msgs   = x[:, src] * edge_w                                     [B, E]
    agg    = scatter_add(msgs -> dst)                               [B, N]
    rate   = base_rate * exp(rate_log_scale)                        [N]
    drive  = tanh(agg + bias)
    x_next = clip(x + DT * rate * (drive*cap - x), 0, cap)

Sharding: destination nodes are dealt round-robin (by total in-degree
rank) across the 8 cores; every edge lives on its destination's core.

Per-core layout (CSC, source-major): the core's edges are grouped by
source node.  Source nodes are dealt (sorted by per-core out-degree,
snake order) across the 128 partitions; each partition's sources are
laid out in equal-out-degree runs padded to a shared (over cores and
partitions) degree curve D.  x rows live in SBUF in dealt order, so
msgs = x * w is a pure Vector-engine broadcast multiply -- no gather.
The per-edge scatter-add into the canonical [nper, 64B] HBM agg buffer
uses the SWDGE dma_scatter_add ucode (CCE read-modify-write adds).
The epilogue reads agg back and applies the bounded ODE step.
"""

import numpy as np

import concourse.bass as bass
import concourse.bacc as bacc
import concourse.mybir as mybir
import concourse.tile as tile
from concourse.bass_utils import run_bass_kernel_spmd

P = 128
NCORES = 8
B = 16
DT = 0.1
ROWE = 64             # agg row stride: 64 f32 = 256B (scatter_add req)
CALL = 8064           # scatter indices per call (2 tx descs/idx ring cap)
NDUMP = 256           # garbage rows for padded slots

F32 = mybir.dt.float32
I16 = mybir.dt.int16
I8 = mybir.dt.int8


def _wrap_idx(flat):
    """SWDGE wrapped idx layout for one call: idx j at [j%16, j//16],
    replicated to 128 partitions (each queue's Q7 pair reads its own 32)."""
    n = len(flat)
    pad = (-n) % 16
    if pad:
        flat = np.concatenate([flat, np.zeros(pad, flat.dtype)])
    cols = flat.reshape(-1, 16).T                  # [16, n/16]
    return np.concatenate([cols] * 8, axis=0)      # [128, n/16]


def _equal_runs(D):
    runs = []
    a = 0
    n = len(D)
    while a < n:
        b = a + 1
        while b < n and D[b] == D[a]:
            b += 1
        if D[a] > 0:
            runs.append((a, b, int(D[a])))
        a = b
    return runs


# ---------------------------------------------------------------------------
# Host-side data preparation
# ---------------------------------------------------------------------------

def _prep(x, theta, bias, ratelog, baserate, cap, sign, conf, delay, src, dst,
          mask, n_cores):
    Bb, N = x.shape
    E = src.shape[0]
    src = np.asarray(src).astype(np.int64)
    dst = np.asarray(dst).astype(np.int64)
    theta = np.asarray(theta, dtype=np.float32)
    sign = np.asarray(sign, dtype=np.float32)
    conf = np.asarray(conf, dtype=np.float32)
    delay = np.asarray(delay, dtype=np.float32)
    mask8 = np.asarray(mask).astype(np.int8)
    xf = np.asarray(x, dtype=np.float32)

    deg = np.bincount(dst, minlength=N)
    order = np.argsort(-deg, kind="stable")
    rank_of = np.empty(N, dtype=np.int64)
    rank_of[order] = np.arange(N)
    core_of = rank_of % n_cores
    pos_of = rank_of // n_cores                    # agg row (p*G+g)
    npc = (N + n_cores - 1) // n_cores
    G = (npc + P - 1) // P
    nper = G * P
    NR = nper + NDUMP

    SRCP = (N + P - 1) // P                        # src ranks per partition

    ec = core_of[dst]
    # per-core out-degree of each source node
    odeg = np.zeros((n_cores, N), np.int32)
    np.add.at(odeg, (ec, src), 1)

    # deal sources (sorted by out-degree, snake) to partitions; build curves
    srcpart = np.empty((n_cores, N), np.int16)
    srcrank = np.empty((n_cores, N), np.int32)
    curves = np.zeros((n_cores, P, SRCP), np.int32)
    dealt_src = np.full((n_cores, P, SRCP), -1, np.int64)
    for c in range(n_cores):
        o = np.argsort(-odeg[c], kind="stable")
        padded = np.full(P * SRCP, -1, np.int64)
        padded[:N] = o
        arr = padded.reshape(SRCP, P)
        arr[1::2] = arr[1::2, ::-1]                # snake
        dealt_src[c] = arr.T                       # [P, SRCP]
        valid = dealt_src[c] >= 0
        sv = dealt_src[c][valid]
        pp_, rr_ = np.nonzero(valid)
        srcpart[c, sv] = pp_.astype(np.int16)
        srcrank[c, sv] = rr_.astype(np.int32)
        curves[c] = np.where(valid, odeg[c][np.maximum(dealt_src[c], 0)], 0)

    D = curves.reshape(-1, SRCP).max(axis=0).astype(np.int64)
    CS = np.zeros(SRCP + 1, np.int64)
    CS[1:] = np.cumsum(D)
    SLOTC = int(CS[-1])
    runs = _equal_runs(D)

    # edge -> slot
    ep = srcpart[ec, src].astype(np.int64)
    er = srcrank[ec, src].astype(np.int64)
    eord = np.lexsort((src, ec))                   # group by (core, src)
    key_change = np.ones(E, dtype=bool)
    key_change[1:] = (ec[eord][1:] != ec[eord][:-1]) | \
                     (src[eord][1:] != src[eord][:-1])
    run_starts = np.flatnonzero(key_change)
    run_id = np.cumsum(key_change) - 1
    occ = np.arange(E) - run_starts[run_id]
    col = CS[er[eord]] + occ
    slot = ep[eord] + P * col                      # linear slot in core

    TOT = SLOTC * P
    ecs = ec[eord]
    sidx = np.empty((n_cores, TOT), np.int16)
    base_dump = (nper + (np.arange(TOT) % NDUMP)).astype(np.int16)
    par_shape = (n_cores, P, SLOTC)
    thetaA = np.zeros(par_shape, np.float32)
    signA = np.zeros(par_shape, np.float32)
    confA = np.zeros(par_shape, np.float32)
    delayA = np.zeros(par_shape, np.float32)
    maskA = np.zeros(par_shape, np.int8)
    for c in range(n_cores):
        sel = ecs == c
        sidx[c] = base_dump
        sidx[c, slot[sel]] = pos_of[dst[eord][sel]].astype(np.int16)
        pidx = (slot[sel] % P, slot[sel] // P)
        thetaA[c][pidx] = theta[eord][sel]
        signA[c][pidx] = sign[eord][sel]
        confA[c][pidx] = conf[eord][sel]
        delayA[c][pidx] = delay[eord][sel]
        maskA[c][pidx] = mask8[eord][sel]

    # passes (node-rank aligned, ~SLOTC/3 cols each) and calls
    npass = 3
    bnd_ranks = [0]
    for i in range(1, npass):
        tgt = SLOTC * i // npass
        r = int(np.searchsorted(CS, tgt))
        bnd_ranks.append(r)
    bnd_ranks.append(SRCP)
    passes = []                                    # (r0, r1, c0, c1)
    for i in range(npass):
        r0, r1 = bnd_ranks[i], bnd_ranks[i + 1]
        passes.append((r0, r1, int(CS[r0]), int(CS[r1])))

    # calls: per pass, chunks of whole columns (<= CALL//P cols)
    calls = []                                     # (c0, ncols) column coords
    for (_, _, c0, c1) in passes:
        a = c0
        while a < c1:
            w_ = min(CALL // P, c1 - a)
            calls.append((a, w_))
            a += w_

    gidx = []
    for c in range(n_cores):
        parts = [_wrap_idx(sidx[c, a * P:(a + w_) * P]) for (a, w_) in calls]
        gidx.append(np.concatenate(parts, axis=1))
    gidx_cols = gidx[0].shape[1]

    # x table in dealt order [P, SRCP*B]
    xcsc = np.zeros((n_cores, P, SRCP, Bb), np.float32)
    for c in range(n_cores):
        v = dealt_src[c] >= 0
        xcsc[c][v] = xf[:, dealt_src[c][v]].T

    # canonical node arrays [P, G]
    def node_arr(vals, fill):
        a = np.full((n_cores, P, G), fill, np.float32)
        for c in range(n_cores):
            posn = np.arange(nper)
            rank = posn * n_cores + c
            ok = rank < N
            nd = order[np.minimum(rank, N - 1)]
            v = np.where(ok, vals[nd], fill).astype(np.float32)
            a[c].reshape(-1)[:] = v                # pos = p*G+g row-major
        return a

    biasA = node_arr(np.asarray(bias, np.float32), 0.0)
    ratelogA = node_arr(np.asarray(ratelog, np.float32), 0.0)
    baserateA = node_arr(np.asarray(baserate, np.float32), 0.0)
    capA = node_arr(np.asarray(cap, np.float32), 1.0)

    xT = xf.T
    xownA = np.zeros((n_cores, P, G, Bb), np.float32)
    node_ids = np.zeros((n_cores, P, G), np.int64)
    for c in range(n_cores):
        posn = np.arange(nper)
        rank = posn * n_cores + c
        ok = rank < N
        nd = np.where(ok, order[np.minimum(rank, N - 1)], -1)
        node_ids[c] = nd.reshape(P, G)
        xownA[c].reshape(nper, Bb)[ok] = xT[nd[ok]]

    ins = []
    for c in range(n_cores):
        ins.append({
            "xcsc": xcsc[c].reshape(P, SRCP * Bb),
            "gidx": gidx[c],
            "theta": thetaA[c],
            "sgn": signA[c],
            "conf": confA[c],
            "delay": delayA[c],
            "maskf": maskA[c],
            "bias": biasA[c],
            "ratelog": ratelogA[c],
            "baserate": baserateA[c],
            "cap": capA[c],
            "xown": xownA[c].reshape(P, G * Bb),
        })
    plan = dict(B=Bb, N=N, G=G, SRCP=SRCP, SLOTC=SLOTC, NR=NR,
                D=D, CS=CS, runs=runs, passes=passes, calls=calls,
                gidx_cols=gidx_cols, node_ids=node_ids)
    return ins, plan


def _assemble(results, plan):
    Bb, N, G = plan["B"], plan["N"], plan["G"]
    out = np.empty((Bb, N), dtype=np.float32)
    for ci, res in enumerate(results):
        o = res["out"].reshape(P, G, Bb)
        nid = plan["node_ids"][ci]
        ok = nid >= 0
        out[:, nid[ok]] = o[ok].T
    return out


# ---------------------------------------------------------------------------
# Device kernel
# ---------------------------------------------------------------------------

def build(Bb, G, SRCP, SLOTC, NR, D, CS, runs, passes, calls, gidx_cols):
    nc = bacc.Bacc("TRN2", target_bir_lowering=False, debug=False,
                   enable_asserts=False, num_swdge_queues=4)

    xcD = nc.dram_tensor("xcsc", [P, SRCP * Bb], F32, kind="ExternalInput")
    giD = nc.dram_tensor("gidx", [128, gidx_cols], I16, kind="ExternalInput")
    thD = nc.dram_tensor("theta", [P, SLOTC], F32, kind="ExternalInput")
    sgD = nc.dram_tensor("sgn", [P, SLOTC], F32, kind="ExternalInput")
    cfD = nc.dram_tensor("conf", [P, SLOTC], F32, kind="ExternalInput")
    dlD = nc.dram_tensor("delay", [P, SLOTC], F32, kind="ExternalInput")
    mkD = nc.dram_tensor("maskf", [P, SLOTC], I8, kind="ExternalInput")
    biD = nc.dram_tensor("bias", [P, G], F32, kind="ExternalInput")
    rlD = nc.dram_tensor("ratelog", [P, G], F32, kind="ExternalInput")
    brD = nc.dram_tensor("baserate", [P, G], F32, kind="ExternalInput")
    cpD = nc.dram_tensor("cap", [P, G], F32, kind="ExternalInput")
    xoD = nc.dram_tensor("xown", [P, G * Bb], F32, kind="ExternalInput")
    outD = nc.dram_tensor("out", [P, G * Bb], F32, kind="ExternalOutput")
    # scatter-add accumulator (zero-initialized by the runtime)
    agD = nc.dram_tensor("agg", [NR, ROWE], F32, kind="ExternalOutput")

    Tanh = mybir.ActivationFunctionType.Tanh
    Exp = mybir.ActivationFunctionType.Exp

    qrr = [0]

    def next_q():
        qrr[0] = (qrr[0] + 1) % 4
        return qrr[0]

    # call index -> offset in gidx (columns of 1/16th slots)
    call_off = [0]
    for (_, w_) in calls:
        call_off.append(call_off[-1] + (w_ * P) // 16)

    with tile.TileContext(nc) as tc:
        with (
            tc.tile_pool(name="persist", bufs=1) as ppool,
            tc.tile_pool(name="work", bufs=2) as wp,
            tc.tile_pool(name="msgs", bufs=2) as mp,
        ):
            xc = ppool.tile([P, SRCP * Bb], F32, tag="xc")
            gi = ppool.tile([128, gidx_cols], I16, tag="gi")
            nc.sync.dma_start(out=xc[:], in_=xcD[:, :])
            nc.sync.dma_start(out=gi[:], in_=giD[:, :])
            x3 = xc[:].rearrange("p (r b) -> p r b", b=Bb)

            ci = 0
            for (r0, r1, c0, c1) in passes:
                pc = c1 - c0
                th = wp.tile([P, pc], F32, tag="th")
                sg = wp.tile([P, pc], F32, tag="sg")
                cf = wp.tile([P, pc], F32, tag="cf")
                dl = wp.tile([P, pc], F32, tag="dl")
                mk = wp.tile([P, pc], I8, tag="mk")
                nc.sync.dma_start(out=th[:], in_=thD[:, c0:c1])
                nc.sync.dma_start(out=sg[:], in_=sgD[:, c0:c1])
                nc.sync.dma_start(out=cf[:], in_=cfD[:, c0:c1])
                nc.sync.dma_start(out=dl[:], in_=dlD[:, c0:c1])
                nc.sync.dma_start(out=mk[:], in_=mkD[:, c0:c1])

                t = wp.tile([P, pc], F32, tag="t")
                w = wp.tile([P, pc], F32, tag="w")
                nc.scalar.activation(t[:], th[:], Tanh)
                nc.vector.tensor_mul(w[:], sg[:], cf[:])
                nc.vector.copy_predicated(w[:], mk[:], t[:])
                nc.vector.tensor_mul(w[:], w[:], dl[:])

                msgs = mp.tile([P, pc * Bb], F32, tag="msgs")
                m3 = msgs[:].rearrange("p (s b) -> p s b", b=Bb)
                # weighted broadcast multiply per equal-degree segment
                for (a, b_, k) in runs:
                    a2, b2 = max(a, r0), min(b_, r1)
                    if a2 >= b2:
                        continue
                    n = b2 - a2
                    oc0 = int(CS[a2]) - c0
                    oc1 = int(CS[b2]) - c0
                    dst_ap = (msgs[:, oc0 * Bb:oc1 * Bb]
                              .rearrange("p (n k b) -> p n k b", k=k, b=Bb))
                    x_ap = (x3[:, a2:b2, :].unsqueeze(2)
                            .to_broadcast([P, n, k, Bb]))
                    w_ap = (w[:, oc0:oc1]
                            .rearrange("p (n k) -> p n k", k=k)
                            .unsqueeze(-1).to_broadcast([P, n, k, Bb]))
                    nc.vector.tensor_tensor(out=dst_ap, in0=x_ap, in1=w_ap,
                                            op=mybir.AluOpType.mult)

                # scatter-add this pass's slots
                while ci < len(calls) and calls[ci][0] < c1:
                    a, w_ = calls[ci]
                    ni = w_ * P
                    nc.gpsimd.dma_scatter_add(
                        agD[:, :Bb],
                        m3[:, a - c0:a - c0 + w_, :],
                        gi[:, call_off[ci]:call_off[ci + 1]],
                        ni, ni, Bb, elem_step=ROWE,
                        single_packet=False, queue_num=next_q())
                    ci += 1

            # ---- readback + ODE epilogue ----
            agg0 = ppool.tile([P, G * Bb], F32, tag="agg0")
            nc.sync.dma_start(
                out=agg0[:].rearrange("p (g b) -> p g b", b=Bb),
                in_=agD[:G * P, :Bb].rearrange("(p g) b -> p g b", p=P))

            bi = ppool.tile([P, G], F32, tag="bi")
            rl = ppool.tile([P, G], F32, tag="rl")
            br = ppool.tile([P, G], F32, tag="br")
            cp = ppool.tile([P, G], F32, tag="cp")
            xo = ppool.tile([P, G * Bb], F32, tag="xo")
            nc.sync.dma_start(out=bi[:], in_=biD[:, :])
            nc.sync.dma_start(out=rl[:], in_=rlD[:, :])
            nc.sync.dma_start(out=br[:], in_=brD[:, :])
            nc.sync.dma_start(out=cp[:], in_=cpD[:, :])
            nc.sync.dma_start(out=xo[:], in_=xoD[:, :])

            rate = ppool.tile([P, G], F32, tag="rate")
            nc.scalar.activation(rate[:], rl[:], Exp)
            nc.vector.tensor_mul(rate[:], rate[:], br[:])

            a3 = agg0[:].rearrange("p (g b) -> p g b", b=Bb)
            bib = bi[:].unsqueeze(-1).to_broadcast([P, G, Bb])
            cpb = cp[:].unsqueeze(-1).to_broadcast([P, G, Bb])
            rateb = rate[:].unsqueeze(-1).to_broadcast([P, G, Bb])

            dr = ppool.tile([P, G * Bb], F32, tag="dr")
            d3 = dr[:].rearrange("p (g b) -> p g b", b=Bb)
            nc.vector.tensor_add(d3, a3, bib)
            nc.scalar.activation(dr[:], dr[:], Tanh)
            nc.vector.tensor_mul(d3, d3, cpb)
            nc.vector.tensor_tensor(out=dr[:], in0=dr[:], in1=xo[:],
                                    op=mybir.AluOpType.subtract)
            nc.vector.tensor_mul(d3, d3, rateb)
            nc.vector.tensor_scalar_mul(dr[:], dr[:], float(DT))
            nc.vector.tensor_add(dr[:], dr[:], xo[:])
            nc.vector.tensor_scalar_max(dr[:], dr[:], 0.0)
            nc.vector.tensor_tensor(out=d3, in0=d3, in1=cpb,
                                    op=mybir.AluOpType.min)
            nc.sync.dma_start(out=outD[:, :], in_=dr[:])

    nc.compile()
    return nc


# ---------------------------------------------------------------------------
# Entry point
# ---------------------------------------------------------------------------

def kernel(x, theta_graph, node_bias, rate_log_scale, base_rate, capacity,
           sign_prior, conf_scale, delay_scale, src_index, dst_index,
           learn_mask):
    ins, plan = _prep(x, theta_graph, node_bias, rate_log_scale, base_rate,
                      capacity, sign_prior, conf_scale, delay_scale,
                      src_index, dst_index, learn_mask, NCORES)
    nc = build(plan["B"], plan["G"], plan["SRCP"], plan["SLOTC"], plan["NR"],
               plan["D"], plan["CS"], plan["runs"], plan["passes"],
               plan["calls"], plan["gidx_cols"])
    res = run_bass_kernel_spmd(nc, ins, core_ids=list(range(NCORES)))
    return _assemble(res.results, plan)


# revision 9
# speedup vs baseline: 1.9525x; 1.4071x over previous
"""Trainium2 Bass kernel for DifferentiableSupergraphDynamics (quad-packed).

Computation:
    edge_w = where(learn_mask, tanh(theta), sign*conf) * delay      [E]
    msgs   = x[:, src] * edge_w                                     [B, E]
    agg    = scatter_add(msgs -> dst)                               [B, N]
    rate   = base_rate * exp(rate_log_scale)                        [N]
    drive  = tanh(agg + bias)
    x_next = clip(x + DT * rate * (drive*cap - x), 0, cap)

Sharding: destination nodes are dealt round-robin (by total in-degree
rank) across the 8 cores; every edge lives on its destination's core.

Per-core edge phase: ONE padded CSR over the core's nodes in canonical
(total in-degree sorted) order, node groups of 128 partitions padded to
the shared-over-cores group max degree D.  Each slot's SWDGE dma_gather
descriptor fetches a 256B quad (4 consecutive x rows, idx = src//4, so
int16 indices reach all 100k rows -- no source-range structures and no
partial-aggregate merge).  A host-built one-hot selector places the edge
weight on the slot's src%4 sub-row and zeroes the other three; the
weighted per-node reduction is a strided Vector-engine tensor_reduce
over the (degree x 4) axis directly into canonical agg order.
"""

import numpy as np

import concourse.bass as bass
import concourse.bacc as bacc
import concourse.mybir as mybir
import concourse.tile as tile
from concourse.bass_utils import run_bass_kernel_spmd

P = 128
NCORES = 8
DT = 0.1
GATHER_CALL = 8192         # SWDGE ring capacity per call
ROWE = 64                  # x row: 64 f32 = 256B quad / 4
QUAD = 4                   # x rows per gather descriptor
CHUNK_COLS = 128           # CSR columns per pass (128*128 slots = 2 calls)

F32 = mybir.dt.float32
I16 = mybir.dt.int16
I8 = mybir.dt.int8


def _wrap_idx(flat, call):
    n = len(flat)
    ncall = (n + call - 1) // call
    pad = ncall * call - n
    if pad:
        flat = np.concatenate([flat, np.zeros(pad, flat.dtype)])
    cols = np.concatenate(
        [flat[k * call:(k + 1) * call].reshape(call // 16, 16).T
         for k in range(ncall)], axis=1)
    return np.concatenate([cols] * 8, axis=0)


# ---------------------------------------------------------------------------
# Host-side data preparation
# ---------------------------------------------------------------------------

def _prep(x, theta, bias, ratelog, baserate, cap, sign, conf, delay, src, dst,
          mask, n_cores):
    B, N = x.shape
    E = src.shape[0]
    src = np.asarray(src).astype(np.int64)
    dst = np.asarray(dst).astype(np.int64)
    theta = np.asarray(theta, dtype=np.float32)
    sign = np.asarray(sign, dtype=np.float32)
    conf = np.asarray(conf, dtype=np.float32)
    delay = np.asarray(delay, dtype=np.float32)
    mask8 = np.asarray(mask).astype(np.int8)

    deg = np.bincount(dst, minlength=N)
    order = np.argsort(-deg, kind="stable")
    npc = (N + n_cores - 1) // n_cores
    G = (npc + P - 1) // P
    nper = G * P

    rank_of = np.empty(N, dtype=np.int64)
    rank_of[order] = np.arange(N)
    core_of = rank_of % n_cores
    pos_of = rank_of // n_cores

    n_pad = ((N + ROWE - 1) // ROWE) * ROWE

    # canonical per-core degree table; shared D curve over cores
    edge_core = core_of[dst]
    edge_pos = pos_of[dst]
    degc = np.zeros((n_cores, nper), dtype=np.int64)
    np.add.at(degc, (edge_core, edge_pos), 1)
    D = np.maximum(degc.reshape(n_cores, G, P).max(axis=2).max(axis=0), 1)
    S = np.zeros(G + 1, dtype=np.int64)
    S[1:] = np.cumsum(D)
    F = int(S[-1])                                  # CSR cols per partition

    # edge -> slot (canonical CSR: node (p,g), occurrence)
    eord = np.lexsort((src, dst))
    ec = edge_core[eord]
    ep = edge_pos[eord]
    key_change = np.ones(E, dtype=bool)
    key_change[1:] = dst[eord][1:] != dst[eord][:-1]
    run_starts = np.flatnonzero(key_change)
    run_id = np.cumsum(key_change) - 1
    occ = np.arange(E) - run_starts[run_id]

    g = ep // P
    pp = ep % P
    col = S[g] + occ
    slot_i = pp + P * col                           # linear slot in core

    par_shape = (n_cores, P, F)
    thetaA = np.zeros(par_shape, np.float32)
    signA = np.zeros(par_shape, np.float32)
    confA = np.zeros(par_shape, np.float32)
    maskA = np.zeros(par_shape, np.int8)
    # delay pre-placed on the slot's src%4 sub-row (0 elsewhere): the
    # device computes w4 = delay4 * where(mask, tanh(theta), sign*conf)
    delay4A = np.zeros((n_cores, P, F * QUAD), np.float32)
    pidx = (ec, pp, col)
    thetaA[pidx] = theta[eord]
    signA[pidx] = sign[eord]
    confA[pidx] = conf[eord]
    maskA[pidx] = mask8[eord]
    delay4A[ec, pp, col * QUAD + (src[eord] % QUAD)] = delay[eord]

    # gather idx per slot: src quad id (uniform 16-wrap: col = slot//16)
    srcq = (src[eord] // QUAD).astype(np.int16)
    TOT = F * P
    gidx = []
    for c in range(n_cores):
        a = np.zeros(TOT, np.int16)
        sel = ec == c
        a[slot_i[sel]] = srcq[sel]
        gidx.append(_wrap_idx(a, 16))
    gidx_cols = gidx[0].shape[1]

    # x quad table: 4 packed 16-f32 rows per 256B table row
    xT4 = np.zeros((n_pad // QUAD, ROWE), np.float32)
    xT4.reshape(n_pad, ROWE // QUAD)[:N, :] = np.asarray(x, np.float32).T

    # node-aligned passes (~CHUNK_COLS cols) with equal-D reduce segments
    passes = []
    ga = 0
    while ga < G:
        gb = ga
        while gb < G and (int(S[gb + 1]) - int(S[ga])) <= CHUNK_COLS:
            gb += 1
        if gb == ga:
            gb = ga + 1                     # single node group wider than chunk
        segs = []
        a = ga
        while a < gb:
            b2 = a + 1
            while b2 < gb and D[b2] == D[a]:
                b2 += 1
            segs.append((a, b2 - a, int(D[a])))
            a = b2
        passes.append((int(S[ga]), int(S[gb]), segs))
        ga = gb

    # canonical CSR node at (p, g) corresponds to pos = g*P + p
    def node_arr2(vals, fill):
        a = np.full((n_cores, P, G), fill, dtype=np.float32)
        for c in range(n_cores):
            posn = np.arange(nper)                 # pos = g*P + p
            rank = posn * n_cores + c
            ok = rank < N
            nd = order[np.minimum(rank, N - 1)]
            v = np.where(ok, vals[nd], fill).astype(np.float32)
            vm = v.reshape(G, P).T                 # [P, G]
            a[c] = vm
        return a

    biasA = node_arr2(np.asarray(bias, np.float32), 0.0)
    ratelogA = node_arr2(np.asarray(ratelog, np.float32), 0.0)
    baserateA = node_arr2(np.asarray(baserate, np.float32), 0.0)
    capA = node_arr2(np.asarray(cap, np.float32), 1.0)

    xT = np.asarray(x, np.float32).T
    xownA = np.zeros((n_cores, P, G, B), np.float32)
    node_ids = np.zeros((n_cores, P, G), np.int64)
    for c in range(n_cores):
        posn = np.arange(nper)
        rank = posn * n_cores + c
        ok = rank < N
        nd = np.where(ok, order[np.minimum(rank, N - 1)], -1)
        node_ids[c] = nd.reshape(G, P).T
        xo = np.zeros((nper, B), np.float32)
        xo[ok] = xT[nd[ok]]
        xownA[c] = xo.reshape(G, P, B).transpose(1, 0, 2)
    ins = []
    for c in range(n_cores):
        ins.append({
            "xT4": xT4,
            "gidx": gidx[c],
            "theta": thetaA[c],
            "sgn": signA[c],
            "conf": confA[c],
            "delay4": delay4A[c],
            "maskf": maskA[c],
            "bias": biasA[c],
            "ratelog": ratelogA[c],
            "baserate": baserateA[c],
            "cap": capA[c],
            "xown": xownA[c].reshape(P, G * B),
        })
    plan = dict(B=B, N=N, G=G, F=F, D=D, S=S, n_pad=n_pad, passes=passes,
                gidx_cols=gidx_cols, node_ids=node_ids)
    return ins, plan


def _assemble(results, plan):
    B, N, G = plan["B"], plan["N"], plan["G"]
    out = np.empty((B, N), dtype=np.float32)
    for ci, res in enumerate(results):
        o = res["out"].reshape(P, G, B)
        nid = plan["node_ids"][ci]
        ok = nid >= 0
        out[:, nid[ok]] = o[ok].T
    return out


# ---------------------------------------------------------------------------
# Device kernel
# ---------------------------------------------------------------------------

def _raw_dma_gather(g, out_ap, in_ap, idxs_ap, num_idxs, elem_size, elem_step,
                    queue_num):
    stride_bytes = elem_step * mybir.dt.size(in_ap.dtype)
    return g.add_instruction(
        mybir.InstDMAGatherAnt(
            name=g.bass.get_next_instruction_name(),
            ins=[*g.lower_ap_dma(in_ap, for_custom_bir_dma=True),
                 g.lower_ap(idxs_ap), g.lower_val_access(g.to_reg(num_idxs))],
            outs=[g.lower_ap(out_ap)],
            transpose=False, num_idxs=num_idxs, elem_size=elem_size,
            stride_bytes_256=stride_bytes // 256, gen_mode=0,
            single_packet=False, queue_num=queue_num,
            sbuf_tokens_per_rank=0, sbuf_free_dim_per_rank=0,
            sbuf_free_dim_pad_per_rank=0, sbuf_byte_offset=0))


def _col_runs(D, c0, c1):
    """Equal-D node runs (in node-group space) covering cols [c0, c1)."""
    S = np.zeros(len(D) + 1, np.int64)
    S[1:] = np.cumsum(D)
    g0 = int(np.searchsorted(S, c0, side="right")) - 1
    runs = []
    a = g0
    while a < len(D) and S[a] < c1:
        b = a + 1
        while b < len(D) and D[b] == D[a] and S[b + 1] <= c1:
            b += 1
        if S[b] > c1 or (b > a and S[b] > c1):
            b = min(b, len(D))
        runs.append((a, b, int(D[a])))
        a = b
    return runs


def build(B, N, G, F, D, S, n_pad, gidx_cols, passes, enable_asserts=False):
    nc = bacc.Bacc("TRN2", target_bir_lowering=False, debug=False,
                   enable_asserts=enable_asserts, num_swdge_queues=4)

    NQROW = n_pad // QUAD
    xT4 = nc.dram_tensor("xT4", [NQROW, ROWE], F32,
                         kind="ExternalInput")
    giD = nc.dram_tensor("gidx", [128, gidx_cols], I16, kind="ExternalInput")
    thD = nc.dram_tensor("theta", [P, F], F32, kind="ExternalInput")
    sgD = nc.dram_tensor("sgn", [P, F], F32, kind="ExternalInput")
    cfD = nc.dram_tensor("conf", [P, F], F32, kind="ExternalInput")
    d4D = nc.dram_tensor("delay4", [P, F * QUAD], F32, kind="ExternalInput")
    mkD = nc.dram_tensor("maskf", [P, F], I8, kind="ExternalInput")
    biD = nc.dram_tensor("bias", [P, G], F32, kind="ExternalInput")
    rlD = nc.dram_tensor("ratelog", [P, G], F32, kind="ExternalInput")
    brD = nc.dram_tensor("baserate", [P, G], F32, kind="ExternalInput")
    cpD = nc.dram_tensor("cap", [P, G], F32, kind="ExternalInput")
    xoD = nc.dram_tensor("xown", [P, G * B], F32, kind="ExternalInput")
    outD = nc.dram_tensor("out", [P, G * B], F32, kind="ExternalOutput")

    Tanh = mybir.ActivationFunctionType.Tanh
    Exp = mybir.ActivationFunctionType.Exp
    BQ = B * QUAD

    qrr = [0]

    def next_q():
        qrr[0] = (qrr[0] + 1) % 4
        return qrr[0]

    with tile.TileContext(nc) as tc:
        with (
            tc.tile_pool(name="persist", bufs=1) as ppool,
            tc.tile_pool(name="work", bufs=2) as wp,
            tc.tile_pool(name="msgs", bufs=2) as mp,
        ):
            # ---- prefetch everything up front ----
            gidx_t = ppool.tile([128, gidx_cols], I16, tag="gidx")
            nc.sync.dma_start(out=gidx_t[:], in_=giD[:, :])
            th = ppool.tile([P, F], F32, tag="th")
            sg = ppool.tile([P, F], F32, tag="sg")
            cf = ppool.tile([P, F], F32, tag="cf")
            d4 = ppool.tile([P, F * QUAD], F32, tag="d4")
            mk = ppool.tile([P, F], I8, tag="mk")
            nc.sync.dma_start(out=th[:], in_=thD[:, :])
            nc.sync.dma_start(out=sg[:], in_=sgD[:, :])
            nc.sync.dma_start(out=cf[:], in_=cfD[:, :])
            nc.sync.dma_start(out=d4[:], in_=d4D[:, :])
            nc.sync.dma_start(out=mk[:], in_=mkD[:, :])
            bi = ppool.tile([P, G], F32, tag="bi")
            rl = ppool.tile([P, G], F32, tag="rl")
            br = ppool.tile([P, G], F32, tag="br")
            cp = ppool.tile([P, G], F32, tag="cp")
            xo = ppool.tile([P, G * B], F32, tag="xo")
            nc.sync.dma_start(out=bi[:], in_=biD[:, :])
            nc.sync.dma_start(out=rl[:], in_=rlD[:, :])
            nc.sync.dma_start(out=br[:], in_=brD[:, :])
            nc.sync.dma_start(out=cp[:], in_=cpD[:, :])
            nc.sync.dma_start(out=xo[:], in_=xoD[:, :])

            # ---- edge weights + quad selector weights ----
            t = ppool.tile([P, F], F32, tag="t")
            w = ppool.tile([P, F], F32, tag="w")
            nc.scalar.activation(t[:], th[:], Tanh)
            nc.vector.tensor_mul(w[:], sg[:], cf[:])
            nc.vector.copy_predicated(w[:], mk[:], t[:])
            w4 = ppool.tile([P, F * QUAD], F32, tag="w4")
            w43 = w4[:].rearrange("p (s q) -> p s q", q=QUAD)
            d43 = d4[:].rearrange("p (s q) -> p s q", q=QUAD)
            wb = w[:].unsqueeze(-1).to_broadcast([P, F, QUAD])
            nc.vector.tensor_tensor(out=w43, in0=d43, in1=wb,
                                    op=mybir.AluOpType.mult)

            agg0 = ppool.tile([P, G * B], F32, tag="agg0")

            in_ap = xT4[:, :]
            # ---- passes over CSR columns (node aligned) ----
            for (c0, c1, segs) in passes:
                pc = c1 - c0
                msgs = mp.tile([P, pc * BQ], F32, tag="msgs")
                m3 = msgs[:].rearrange("p (s e) -> p s e", e=BQ)
                tot = pc * P
                nca = (tot + GATHER_CALL - 1) // GATHER_CALL
                gb = (c0 * P) // 16
                for k in range(nca):
                    i0 = k * GATHER_CALL
                    ni = min(GATHER_CALL, tot - i0)
                    _raw_dma_gather(
                        nc.gpsimd,
                        m3[:, i0 // P:(i0 + ni) // P, :],
                        in_ap,
                        gidx_t[:, gb + i0 // 16:gb + (i0 + ni) // 16],
                        ni, BQ, ROWE, next_q())

                # weight: msgs[p, s, q, b] *= w4[p, s, q]
                m4 = msgs[:].rearrange("p (s q b) -> p s q b", q=QUAD, b=B)
                w4b = (w4[:, c0 * QUAD:c1 * QUAD]
                       .rearrange("p (s q) -> p s q", q=QUAD)
                       .unsqueeze(-1).to_broadcast([P, pc, QUAD, B]))
                nc.vector.tensor_tensor(out=m4, in0=m4, in1=w4b,
                                        op=mybir.AluOpType.mult)

                # reduce per equal-D node segment into canonical agg
                for (na, n, d) in segs:
                    if d == 0:
                        continue
                    lo = int(S[na])
                    src_ap = (msgs[:, (lo - c0) * BQ:
                                   (lo - c0 + n * d) * BQ]
                              .rearrange("p (n k b) -> p n b k",
                                         n=n, k=d * QUAD, b=B))
                    dst_ap = agg0[:, na * B:(na + n) * B].rearrange(
                        "p (n b) -> p n b", b=B)
                    nc.vector.tensor_reduce(
                        dst_ap, src_ap, axis=mybir.AxisListType.X,
                        op=mybir.AluOpType.add)

            # ---- ODE epilogue ----
            rate = ppool.tile([P, G], F32, tag="rate")
            nc.scalar.activation(rate[:], rl[:], Exp)
            nc.vector.tensor_mul(rate[:], rate[:], br[:])

            a3 = agg0[:].rearrange("p (g b) -> p g b", b=B)
            bib = bi[:].unsqueeze(-1).to_broadcast([P, G, B])
            cpb = cp[:].unsqueeze(-1).to_broadcast([P, G, B])
            rateb = rate[:].unsqueeze(-1).to_broadcast([P, G, B])

            dr = ppool.tile([P, G * B], F32, tag="dr")
            d3 = dr[:].rearrange("p (g b) -> p g b", b=B)
            nc.vector.tensor_add(d3, a3, bib)
            nc.scalar.activation(dr[:], dr[:], Tanh)
            nc.vector.tensor_mul(d3, d3, cpb)
            nc.vector.tensor_tensor(out=dr[:], in0=dr[:], in1=xo[:],
                                    op=mybir.AluOpType.subtract)
            nc.vector.tensor_mul(d3, d3, rateb)
            nc.vector.tensor_scalar_mul(dr[:], dr[:], float(DT))
            nc.vector.tensor_add(dr[:], dr[:], xo[:])
            nc.vector.tensor_scalar_max(dr[:], dr[:], 0.0)
            nc.vector.tensor_tensor(out=d3, in0=d3, in1=cpb,
                                    op=mybir.AluOpType.min)
            nc.sync.dma_start(out=outD[:, :], in_=dr[:])

    nc.compile()
    return nc


# ---------------------------------------------------------------------------
# Entry point
# ---------------------------------------------------------------------------

def kernel(x, theta_graph, node_bias, rate_log_scale, base_rate, capacity,
           sign_prior, conf_scale, delay_scale, src_index, dst_index,
           learn_mask):
    ins, plan = _prep(x, theta_graph, node_bias, rate_log_scale, base_rate,
                      capacity, sign_prior, conf_scale, delay_scale,
                      src_index, dst_index, learn_mask, NCORES)
    nc = build(plan["B"], plan["N"], plan["G"], plan["F"], plan["D"],
               plan["S"], plan["n_pad"], plan["gidx_cols"], plan["passes"])
    res = run_bass_kernel_spmd(nc, ins, core_ids=list(range(NCORES)))
    return _assemble(res.results, plan)


# revision 11
# speedup vs baseline: 2.0978x; 1.0744x over previous
"""Trainium2 Bass kernel for DifferentiableSupergraphDynamics.

Computation:
    edge_w = where(learn_mask, tanh(theta), sign*conf) * delay      [E]
    msgs   = x[:, src] * edge_w                                     [B, E]
    agg    = scatter_add(msgs -> dst)                               [B, N]
    rate   = base_rate * exp(rate_log_scale)                        [N]
    drive  = tanh(agg + bias)
    x_next = clip(x + DT * rate * (drive*cap - x), 0, cap)

Sharding: destination nodes are dealt round-robin (by total in-degree
rank) across the 8 cores; every edge lives on its destination's core, so
no cross-core collective is needed.

Per-core edge phase: edges are split into (up to) 4 "structures" by
source-node range (32768 rows each, so dma_gather's int16 indices can
address the x table). Each structure is a padded CSR over the core's
nodes sorted by that structure's in-degree: node groups of 128
partitions padded to the group max degree D. The x-row gather for all of
a structure's slots is done with the vectorized SWDGE dma_gather ucode
(one 64B descriptor per slot, round-robin over the 4 SWDGE queues), the
weighted per-node reduction is a strided Vector-engine tensor_reduce,
and the 4 per-structure partial aggregates are merged into structure-0's
node order with unique-index dma_scatter_add through HBM.
"""

import numpy as np

import concourse.bass as bass
import concourse.bacc as bacc
import concourse.mybir as mybir
import concourse.tile as tile
from concourse.bass_utils import run_bass_kernel_spmd

P = 128
NCORES = 8
DT = 0.1
SRC_CHUNK = 32768          # dma_gather int16 index reach
GATHER_CALL = 8192         # SWDGE ring capacity per call
XBF16 = False              # f32 x rows (64B descriptors); bf16 trips clip-boundary rel err
ROWE = 64                  # x-table row stride: 256B (dma_gather req)
XROW = 128 if XBF16 else 64

F32 = mybir.dt.float32
I16 = mybir.dt.int16
I8 = mybir.dt.int8


def _wrap_idx(flat, call):
    """Lay a flat int16 index list out in the SWDGE wrapped layout:
    per call of `call` indices, index j at [j%16, j//16]; 32-partition
    replicated (descriptor-gen runs on two Q7 cores)."""
    n = len(flat)
    ncall = (n + call - 1) // call
    pad = ncall * call - n
    if pad:
        flat = np.concatenate([flat, np.zeros(pad, flat.dtype)])
    cols = np.concatenate(
        [flat[k * call:(k + 1) * call].reshape(call // 16, 16).T
         for k in range(ncall)], axis=1)          # [16, ncall*call/16]
    return np.concatenate([cols] * 8, axis=0)     # [128, ...]


# ---------------------------------------------------------------------------
# Host-side data preparation
# ---------------------------------------------------------------------------

def _prep(x, theta, bias, ratelog, baserate, cap, sign, conf, delay, src, dst,
          mask, n_cores):
    B, N = x.shape
    E = src.shape[0]

    src = np.asarray(src).astype(np.int64)
    dst = np.asarray(dst).astype(np.int64)
    theta = np.asarray(theta, dtype=np.float32)
    sign = np.asarray(sign, dtype=np.float32)
    conf = np.asarray(conf, dtype=np.float32)
    delay = np.asarray(delay, dtype=np.float32)
    mask8 = np.asarray(mask).astype(np.int8)

    deg = np.bincount(dst, minlength=N)
    order = np.argsort(-deg, kind="stable")
    npc = (N + n_cores - 1) // n_cores
    G = (npc + P - 1) // P
    nper = G * P                                   # nodes per core (padded)

    rank_of = np.empty(N, dtype=np.int64)
    rank_of[order] = np.arange(N)
    core_of = rank_of % n_cores                    # node -> core
    pos_of = rank_of // n_cores                    # node -> position in core

    n_pad = ((N + ROWE - 1) // ROWE) * ROWE
    nq = (n_pad + SRC_CHUNK - 1) // SRC_CHUNK     # structures
    q_of = src // SRC_CHUNK                        # edge -> structure

    # per (core, structure) in-degree
    edge_core = core_of[dst]
    edge_pos = pos_of[dst]
    degq = np.zeros((n_cores, nper, nq), dtype=np.int64)
    np.add.at(degq, (edge_core, edge_pos, q_of), 1)

    # shared-over-cores placement per structure: within each core sort
    # positions by degq desc; group windows of 128; D = max over cores.
    D = np.zeros((nq, G), dtype=np.int64)
    ordq = np.zeros((n_cores, nq, nper), dtype=np.int64)   # row j -> position
    invq = np.zeros((n_cores, nq, nper), dtype=np.int64)   # position -> row j
    for q in range(nq):
        for c in range(n_cores):
            o = np.argsort(-degq[c, :, q], kind="stable")
            ordq[c, q] = o
            invq[c, q, o] = np.arange(nper)
            dm = degq[c, o, q].reshape(G, P).max(axis=1)
            D[q] = np.maximum(D[q], dm)
    D[0] = np.maximum(D[0], 1)       # canonical layout covers all nodes
    S = np.zeros((nq, G + 1), dtype=np.int64)
    S[:, 1:] = np.cumsum(D, axis=1)
    F = S[:, -1]                                   # cols per structure
    Gact = np.array([int((D[q] > 0).sum()) for q in range(nq)])

    # --- edge slot assignment ---
    # edge -> (core, structure, row=invq, occurrence within (node,structure))
    eord = np.lexsort((src, dst))                  # group by dst, then src q
    ec = edge_core[eord]
    ep = edge_pos[eord]
    eq = q_of[eord]
    # occurrence counter within (dst, q): edges sorted by (dst, q)
    key_change = np.ones(E, dtype=bool)
    key_change[1:] = (dst[eord][1:] != dst[eord][:-1]) | (eq[1:] != eq[:-1])
    run_id = np.cumsum(key_change) - 1
    run_starts = np.flatnonzero(key_change)
    occ = np.arange(E) - run_starts[run_id]

    row = invq[ec, eq, ep]                         # row index in structure
    g = row // P
    pp = row % P
    col = S[eq, g] + occ
    # slot linear index within (core, structure): i = pp + 128*col
    slot_i = pp + P * col

    # per (core, structure) arrays
    ins = []
    FT = int(F.sum())                              # total cols, all structures
    Scol = np.zeros(nq + 1, dtype=np.int64)
    Scol[1:] = np.cumsum(F)

    # params laid out [P, FT] per core (slot (q,p,col) -> [p, Scol[q]+col])
    par_shape = (n_cores, P, FT)
    thetaA = np.zeros(par_shape, np.float32)
    signA = np.zeros(par_shape, np.float32)
    confA = np.zeros(par_shape, np.float32)
    delayA = np.zeros(par_shape, np.float32)
    maskA = np.zeros(par_shape, np.int8)
    pidx = (ec, pp, Scol[eq] + col)
    thetaA[pidx] = theta[eord]
    signA[pidx] = sign[eord]
    confA[pidx] = conf[eord]
    delayA[pidx] = delay[eord]
    maskA[pidx] = mask8[eord]

    # gather index lists (wrapped) per core: concat over structures of
    # the per-structure slot-order index list (idx relative to q base)
    srcrel = (src[eord] - eq * SRC_CHUNK).astype(np.int16)
    gidx = []
    ncalls = np.zeros(nq, dtype=np.int64)
    for c in range(n_cores):
        parts = []
        for q in range(nq):
            tot = int(F[q]) * P
            a = np.zeros(tot, np.int16)
            selq = (ec == c) & (eq == q)
            a[slot_i[selq]] = srcrel[selq]
            parts.append(_wrap_idx(a, GATHER_CALL))
            ncalls[q] = (tot + GATHER_CALL - 1) // GATHER_CALL
        gidx.append(np.concatenate(parts, axis=1))
    gidx_cols = gidx[0].shape[1]

    # canonical placement = structure 0's; scatter index for structures
    # 1..nq-1: input position i = p + 128*g -> canonical slot p0*G + g0
    sidx = []
    for c in range(n_cores):
        parts = []
        for q in range(1, nq):
            node_pos = ordq[c, q]                  # row j -> position
            crow = invq[c, 0, node_pos]            # canonical row of node
            canon = (crow % P) * G + (crow // P)   # partition-major slot id
            a = np.zeros(nper, np.int16)
            jj = np.arange(nper)
            a[(jj % P) + P * (jj // P)] = canon.astype(np.int16)
            nact = int(Gact[q]) * P
            half = min((G // 2) * P, nact)
            parts.append(_wrap_idx(a[:half], half))
            if nact > half:
                parts.append(_wrap_idx(a[half:nact], nact - half))
        sidx.append(np.concatenate(parts, axis=1) if parts else
                    np.zeros((128, 16), np.int16))
    sidx_cols = sidx[0].shape[1]

    # node params in canonical placement [P, G]
    def node_arr(vals, fill):
        a = np.full((n_cores, P, G), fill, dtype=np.float32)
        for c in range(n_cores):
            node_pos = ordq[c, 0]                  # canonical row j -> pos
            rank = node_pos * n_cores + c          # position -> rank
            ok = rank < N
            nd = order[np.minimum(rank, N - 1)]
            v = np.where(ok, vals[nd], fill).astype(np.float32)
            a[c].reshape(-1)[(np.arange(nper) % P) * G +
                             (np.arange(nper) // P)] = np.where(
                                 ok, v, fill)
        return a

    biasA = node_arr(np.asarray(bias), 0.0)
    ratelogA = node_arr(np.asarray(ratelog), 0.0)
    baserateA = node_arr(np.asarray(baserate), 0.0)
    capA = node_arr(np.asarray(cap), 1.0)

    import ml_dtypes
    xdt = ml_dtypes.bfloat16 if XBF16 else np.float32
    xT4 = np.zeros((n_pad, XROW), xdt)
    xT4[:N, :B] = np.asarray(x, dtype=np.float32).T.astype(xdt)

    xTf = np.asarray(x, dtype=np.float32).T
    xownA = np.zeros((n_cores, P, G, B), np.float32)
    node_ids = np.zeros((n_cores, P, G), np.int64)
    for c in range(n_cores):
        node_pos = ordq[c, 0]
        rank = node_pos * n_cores + c
        ok = rank < N
        nd = np.where(ok, order[np.minimum(rank, N - 1)], -1)
        jj = np.arange(nper)
        pcol = (jj % P, jj // P)
        node_ids[c][pcol] = nd
        xownA[c][pcol[0], pcol[1], :] = np.where(
            ok[:, None], xTf[np.maximum(nd, 0), :], 0.0)

    for c in range(n_cores):
        ins.append({
            "xT4": xT4,
            "gidx": gidx[c],
            "sidx": sidx[c],
            "theta": thetaA[c],
            "sgn": signA[c],
            "conf": confA[c],
            "delay": delayA[c],
            "maskf": maskA[c],
            "bias": biasA[c],
            "ratelog": ratelogA[c],
            "baserate": baserateA[c],
            "cap": capA[c],
            "xown": xownA[c].reshape(P, G * B),
        })
    plan = dict(B=B, N=N, G=G, nq=nq, D=D, S=S, F=F, Scol=Scol, Gact=Gact,
                n_pad=n_pad, gidx_cols=gidx_cols, sidx_cols=sidx_cols,
                node_ids=node_ids)
    return ins, plan


def _assemble(results, plan):
    B, N, G = plan["B"], plan["N"], plan["G"]
    out = np.empty((B, N), dtype=np.float32)
    for ci, res in enumerate(results):
        o = res["out"].reshape(P, G, B)
        nid = plan["node_ids"][ci]
        ok = nid >= 0
        out[:, nid[ok]] = o[ok].T
    return out


# ---------------------------------------------------------------------------
# Device kernel
# ---------------------------------------------------------------------------

def _raw_dma_gather(g, out_ap, in_ap, idxs_ap, num_idxs, elem_size, elem_step,
                    queue_num):
    stride_bytes = elem_step * mybir.dt.size(in_ap.dtype)
    return g.add_instruction(
        mybir.InstDMAGatherAnt(
            name=g.bass.get_next_instruction_name(),
            ins=[*g.lower_ap_dma(in_ap, for_custom_bir_dma=True),
                 g.lower_ap(idxs_ap), g.lower_val_access(g.to_reg(num_idxs))],
            outs=[g.lower_ap(out_ap)],
            transpose=False, num_idxs=num_idxs, elem_size=elem_size,
            stride_bytes_256=stride_bytes // 256, gen_mode=0,
            single_packet=False, queue_num=queue_num,
            sbuf_tokens_per_rank=0, sbuf_free_dim_per_rank=0,
            sbuf_free_dim_pad_per_rank=0, sbuf_byte_offset=0))


def _equal_d_runs(D, g0, g1):
    runs = []
    a = g0
    while a < g1:
        b = a + 1
        while b < g1 and D[b] == D[a]:
            b += 1
        runs.append((a, b, int(D[a])))
        a = b
    return runs


def build(B, N, G, nq, D, S, F, Scol, n_pad, gidx_cols, sidx_cols,
          Gact=None, node_ids=None, enable_asserts=False, loop_r=None):
    if Gact is None:
        Gact = np.array([G] * nq)
    FT = int(Scol[-1])
    nc = bacc.Bacc("TRN2", target_bir_lowering=False, debug=False,
                   enable_asserts=enable_asserts, num_swdge_queues=4)

    XDT = mybir.dt.bfloat16 if XBF16 else F32
    xT4 = nc.dram_tensor("xT4", [n_pad, XROW], XDT, kind="ExternalInput")
    giD = nc.dram_tensor("gidx", [128, gidx_cols], I16, kind="ExternalInput")
    siD = nc.dram_tensor("sidx", [128, sidx_cols], I16, kind="ExternalInput")
    thD = nc.dram_tensor("theta", [P, FT], F32, kind="ExternalInput")
    sgD = nc.dram_tensor("sgn", [P, FT], F32, kind="ExternalInput")
    cfD = nc.dram_tensor("conf", [P, FT], F32, kind="ExternalInput")
    dlD = nc.dram_tensor("delay", [P, FT], F32, kind="ExternalInput")
    mkD = nc.dram_tensor("maskf", [P, FT], I8, kind="ExternalInput")
    biD = nc.dram_tensor("bias", [P, G], F32, kind="ExternalInput")
    rlD = nc.dram_tensor("ratelog", [P, G], F32, kind="ExternalInput")
    brD = nc.dram_tensor("baserate", [P, G], F32, kind="ExternalInput")
    cpD = nc.dram_tensor("cap", [P, G], F32, kind="ExternalInput")
    xoD = nc.dram_tensor("xown", [P, G * B], F32, kind="ExternalInput")
    outD = nc.dram_tensor("out", [P, G * B], F32, kind="ExternalOutput")
    # partial-agg merge buffers (zero-initialized by the runtime)
    pagg = [nc.dram_tensor(f"pagg{q}", [G * P, ROWE], F32,
                           kind="ExternalOutput") for q in range(1, nq)]

    Tanh = mybir.ActivationFunctionType.Tanh
    Exp = mybir.ActivationFunctionType.Exp

    qrr = [0]

    def next_q():
        qrr[0] = (qrr[0] + 1) % 4
        return qrr[0]

    import contextlib
    with tile.TileContext(nc) as tc:
        with (
            tc.tile_pool(name="persist", bufs=1) as ppool,
            tc.tile_pool(name="work", bufs=2) as wp,
            tc.tile_pool(name="msgs", bufs=2) as mp,
        ):
          with (tc.For_i(0, loop_r, 1) if loop_r else
                contextlib.nullcontext()):
            agg0 = ppool.tile([P, G * B], F32, tag="agg0")
            aggq_tiles = []

            qorder = list(range(1, nq)) + [0]
            gidx_bases = np.zeros(nq + 1, dtype=np.int64)
            sidx_bases = np.zeros(nq, dtype=np.int64)
            sb = 0
            for q in range(nq):
                tot = int(F[q]) * P
                ncall = (tot + GATHER_CALL - 1) // GATHER_CALL if tot else 0
                gidx_bases[q + 1] = gidx_bases[q] + ncall * (GATHER_CALL // 16)
                if q >= 1:
                    sidx_bases[q] = sb
                    nact = int(Gact[q]) * P
                    sb += (nact + 15) // 16
            # ---- prefetch all idx/param/node streams up front ----
            gidx_t = ppool.tile([128, int(gidx_bases[nq])], I16, tag="gidx")
            nc.sync.dma_start(out=gidx_t[:], in_=giD[:, :])
            pt = {}
            for q in qorder:
                Fq = int(F[q])
                if Fq == 0:
                    continue
                c0, c1 = int(Scol[q]), int(Scol[q + 1])
                th = ppool.tile([P, Fq], F32, tag=f"th{q}")
                sg = ppool.tile([P, Fq], F32, tag=f"sg{q}")
                cf = ppool.tile([P, Fq], F32, tag=f"cf{q}")
                dl = ppool.tile([P, Fq], F32, tag=f"dl{q}")
                mk = ppool.tile([P, Fq], I8, tag=f"mk{q}")
                nc.sync.dma_start(out=th[:], in_=thD[:, c0:c1])
                nc.sync.dma_start(out=sg[:], in_=sgD[:, c0:c1])
                nc.sync.dma_start(out=cf[:], in_=cfD[:, c0:c1])
                nc.sync.dma_start(out=dl[:], in_=dlD[:, c0:c1])
                nc.sync.dma_start(out=mk[:], in_=mkD[:, c0:c1])
                pt[q] = (th, sg, cf, dl, mk)
            bi = ppool.tile([P, G], F32, tag="bi")
            rl = ppool.tile([P, G], F32, tag="rl")
            br = ppool.tile([P, G], F32, tag="br")
            cp = ppool.tile([P, G], F32, tag="cp")
            xo = ppool.tile([P, G * B], F32, tag="xo")
            nc.sync.dma_start(out=bi[:], in_=biD[:, :])
            nc.sync.dma_start(out=rl[:], in_=rlD[:, :])
            nc.sync.dma_start(out=br[:], in_=brD[:, :])
            nc.sync.dma_start(out=cp[:], in_=cpD[:, :])
            nc.sync.dma_start(out=xo[:], in_=xoD[:, :])

            for q in qorder:
                Fq = int(F[q])
                TOTq = Fq * P
                if TOTq == 0:
                    continue
                ncall = (TOTq + GATHER_CALL - 1) // GATHER_CALL
                gb = int(gidx_bases[q])

                (th, sg, cf, dl, mk) = pt[q]
                t = wp.tile([P, Fq], F32, tag="t")
                w = wp.tile([P, Fq], F32, tag="w")
                nc.scalar.activation(t[:], th[:], Tanh)
                nc.vector.tensor_mul(w[:], sg[:], cf[:])
                nc.vector.copy_predicated(w[:], mk[:], t[:])
                nc.vector.tensor_mul(w[:], w[:], dl[:])

                msgs = mp.tile([P, Fq * B], F32, tag="msgs")
                m3 = msgs[:].rearrange("p (s b) -> p s b", b=B)
                base = q * SRC_CHUNK
                in_ap = xT4[base:min(base + SRC_CHUNK, n_pad), :B]
                if XBF16:
                    msgsr = mp.tile([P, Fq * B], XDT, tag="msgsr")
                    gdst = msgsr[:].rearrange("p (s b) -> p s b", b=B)
                else:
                    gdst = m3
                for k in range(ncall):
                    i0 = k * GATHER_CALL
                    ni = min(GATHER_CALL, TOTq - i0)
                    _raw_dma_gather(
                        nc.gpsimd,
                        gdst[:, i0 // P:(i0 + ni) // P, :],
                        in_ap,
                        gidx_t[:, gb + k * (GATHER_CALL // 16):
                               gb + k * (GATHER_CALL // 16) + (ni + 15) // 16],
                        ni, B, XROW, next_q())

                wb = w[:].unsqueeze(-1).to_broadcast([P, Fq, B])
                if XBF16:
                    nc.vector.tensor_tensor(out=m3, in0=gdst, in1=wb,
                                            op=mybir.AluOpType.mult)
                else:
                    nc.vector.tensor_mul(m3, m3, wb)

                if q == 0:
                    aggt = agg0
                else:
                    aggt = wp.tile([P, G * B], F32, tag="aggq")
                    aggq_tiles.append(aggt)
                for (ga, gb2, d) in _equal_d_runs(D[q], 0, int(Gact[q])):
                    if d == 0:
                        continue
                    src_ap = (m3[:, int(S[q, ga]):int(S[q, gb2]), :]
                              .rearrange("p (n d) b -> p n b d", d=d))
                    dst_ap = aggt[:, ga * B:gb2 * B].rearrange(
                        "p (n b) -> p n b", b=B)
                    nc.vector.tensor_reduce(
                        dst_ap, src_ap, axis=mybir.AxisListType.X,
                        op=mybir.AluOpType.add)

                if q > 0:
                    # scatter active rows into canonical order through HBM
                    a3 = aggt[:].rearrange("p (g b) -> p g b", b=B)
                    nact = int(Gact[q]) * P
                    half = min((G // 2) * P, nact)
                    sbase = int(sidx_bases[q])
                    nc.gpsimd.dma_scatter_add(
                        pagg[q - 1][:, :B], a3[:, :half // P, :],
                        _slice_idx(wp, nc, siD, sbase, half),
                        half, half, B, elem_step=ROWE,
                        single_packet=False, queue_num=next_q())
                    if nact > half:
                        nc.gpsimd.dma_scatter_add(
                            pagg[q - 1][:, :B],
                            a3[:, half // P:nact // P, :],
                            _slice_idx(wp, nc, siD, sbase + half // 16,
                                       nact - half),
                            nact - half, nact - half, B, elem_step=ROWE,
                            single_packet=False, queue_num=next_q())

            # ---- merge + ODE epilogue ----
            rdb = []
            for q in range(1, nq):
                if int(Gact[q]) == 0:
                    continue
                rt = ppool.tile([P, G * B], F32, tag=f"rdb{q}")
                nc.sync.dma_start(
                    out=rt[:].rearrange("p (g b) -> p g b", b=B),
                    in_=pagg[q - 1][:, :B].rearrange(
                        "(p g) b -> p g b", p=P))
                rdb.append(rt)
            for rt in rdb:
                nc.vector.tensor_add(agg0[:], agg0[:], rt[:])

            rate = ppool.tile([P, G], F32, tag="rate")
            nc.scalar.activation(rate[:], rl[:], Exp)
            nc.vector.tensor_mul(rate[:], rate[:], br[:])

            a3 = agg0[:].rearrange("p (g b) -> p g b", b=B)
            bib = bi[:].unsqueeze(-1).to_broadcast([P, G, B])
            cpb = cp[:].unsqueeze(-1).to_broadcast([P, G, B])
            rateb = rate[:].unsqueeze(-1).to_broadcast([P, G, B])

            dr = ppool.tile([P, G * B], F32, tag="dr")
            d3 = dr[:].rearrange("p (g b) -> p g b", b=B)
            nc.vector.tensor_add(d3, a3, bib)
            nc.scalar.activation(dr[:], dr[:], Tanh)
            nc.vector.tensor_mul(d3, d3, cpb)
            nc.vector.tensor_tensor(out=dr[:], in0=dr[:], in1=xo[:],
                                    op=mybir.AluOpType.subtract)
            nc.vector.tensor_mul(d3, d3, rateb)
            nc.vector.tensor_scalar_mul(dr[:], dr[:], float(DT))
            nc.vector.tensor_add(dr[:], dr[:], xo[:])
            nc.vector.tensor_scalar_max(dr[:], dr[:], 0.0)
            nc.vector.tensor_tensor(out=d3, in0=d3, in1=cpb,
                                    op=mybir.AluOpType.min)
            nc.sync.dma_start(out=outD[:, :], in_=dr[:])

    nc.compile()
    return nc


def _slice_idx(wp, nc, siD, col0, n):
    t = wp.tile([128, (n + 15) // 16], I16, tag="sidx")
    nc.sync.dma_start(out=t[:], in_=siD[:, col0:col0 + (n + 15) // 16])
    return t[:]


# ---------------------------------------------------------------------------
# Entry point
# ---------------------------------------------------------------------------

def kernel(x, theta_graph, node_bias, rate_log_scale, base_rate, capacity,
           sign_prior, conf_scale, delay_scale, src_index, dst_index,
           learn_mask):
    ins, plan = _prep(x, theta_graph, node_bias, rate_log_scale, base_rate,
                      capacity, sign_prior, conf_scale, delay_scale,
                      src_index, dst_index, learn_mask, NCORES)
    nc = build(plan["B"], plan["N"], plan["G"], plan["nq"], plan["D"],
               plan["S"], plan["F"], plan["Scol"], plan["n_pad"],
               plan["gidx_cols"], plan["sidx_cols"], Gact=plan["Gact"])
    res = run_bass_kernel_spmd(nc, ins, core_ids=list(range(NCORES)))
    return _assemble(res.results, plan)



# revision 15
# speedup vs baseline: 2.6306x; 1.2540x over previous
"""Trainium2 Bass kernel for DifferentiableSupergraphDynamics.

Computation:
    edge_w = where(learn_mask, tanh(theta), sign*conf) * delay      [E]
    msgs   = x[:, src] * edge_w                                     [B, E]
    agg    = scatter_add(msgs -> dst)                               [B, N]
    rate   = base_rate * exp(rate_log_scale)                        [N]
    drive  = tanh(agg + bias)
    x_next = clip(x + DT * rate * (drive*cap - x), 0, cap)

Sharding: destination nodes are dealt round-robin (by total in-degree
rank) across the 8 cores; every edge lives on its destination's core, so
no cross-core collective is needed.

Per-core edge phase: edges are split into (up to) 4 "structures" by
source-node range (32768 rows each, so dma_gather's int16 indices can
address the x table). Each structure is a padded CSR over the core's
nodes sorted by that structure's in-degree: node groups of 128
partitions padded to the group max degree D. The x-row gather for all of
a structure's slots is done with the vectorized SWDGE dma_gather ucode
(one 64B descriptor per slot, round-robin over the 4 SWDGE queues), the
weighted per-node reduction is a strided Vector-engine tensor_reduce,
and the 4 per-structure partial aggregates are merged into structure-0's
node order with unique-index dma_scatter_add through HBM.
"""

import numpy as np

import concourse.bass as bass
import concourse.bacc as bacc
import concourse.mybir as mybir
import concourse.tile as tile
from concourse.bass_utils import run_bass_kernel_spmd

P = 128
NCORES = 8
DT = 0.1
SRC_CHUNK = 32768          # dma_gather int16 index reach
GATHER_CALL = 8064         # 63*128: call-aligned, 505 ring descs (2 per ring)
XBF16 = False              # f32 x rows (64B descriptors); bf16 trips clip-boundary rel err
ROWE = 64                  # x-table row stride: 256B (dma_gather req)
XROW = 128 if XBF16 else 64

F32 = mybir.dt.float32
I16 = mybir.dt.int16
I8 = mybir.dt.int8


def _wrap_idx(flat, call):
    """Lay a flat int16 index list out in the SWDGE wrapped layout:
    per call of `call` indices, index j at [j%16, j//16]; 32-partition
    replicated (descriptor-gen runs on two Q7 cores)."""
    n = len(flat)
    ncall = (n + call - 1) // call
    pad = ncall * call - n
    if pad:
        flat = np.concatenate([flat, np.zeros(pad, flat.dtype)])
    cols = np.concatenate(
        [flat[k * call:(k + 1) * call].reshape(call // 16, 16).T
         for k in range(ncall)], axis=1)          # [16, ncall*call/16]
    return np.concatenate([cols] * 8, axis=0)     # [128, ...]


# ---------------------------------------------------------------------------
# Host-side data preparation
# ---------------------------------------------------------------------------

def _prep(x, theta, bias, ratelog, baserate, cap, sign, conf, delay, src, dst,
          mask, n_cores):
    B, N = x.shape
    E = src.shape[0]

    src = np.asarray(src).astype(np.int64)
    dst = np.asarray(dst).astype(np.int64)
    theta = np.asarray(theta, dtype=np.float32)
    sign = np.asarray(sign, dtype=np.float32)
    conf = np.asarray(conf, dtype=np.float32)
    delay = np.asarray(delay, dtype=np.float32)
    mask8 = np.asarray(mask).astype(np.int8)

    deg = np.bincount(dst, minlength=N)
    order = np.argsort(-deg, kind="stable")
    npc = (N + n_cores - 1) // n_cores
    G = (npc + P - 1) // P
    nper = G * P                                   # nodes per core (padded)

    rank_of = np.empty(N, dtype=np.int64)
    rank_of[order] = np.arange(N)
    core_of = rank_of % n_cores                    # node -> core
    pos_of = rank_of // n_cores                    # node -> position in core

    n_pad = ((N + ROWE - 1) // ROWE) * ROWE
    nq = (n_pad + SRC_CHUNK - 1) // SRC_CHUNK     # structures
    q_of = src // SRC_CHUNK                        # edge -> structure

    # per (core, structure) in-degree
    edge_core = core_of[dst]
    edge_pos = pos_of[dst]
    degq = np.zeros((n_cores, nper, nq), dtype=np.int64)
    np.add.at(degq, (edge_core, edge_pos, q_of), 1)

    # shared-over-cores placement per structure: within each core sort
    # positions by degq desc; group windows of 128; D = max over cores.
    D = np.zeros((nq, G), dtype=np.int64)
    ordq = np.zeros((n_cores, nq, nper), dtype=np.int64)   # row j -> position
    invq = np.zeros((n_cores, nq, nper), dtype=np.int64)   # position -> row j
    for q in range(nq):
        for c in range(n_cores):
            o = np.argsort(-degq[c, :, q], kind="stable")
            ordq[c, q] = o
            invq[c, q, o] = np.arange(nper)
            dm = degq[c, o, q].reshape(G, P).max(axis=1)
            D[q] = np.maximum(D[q], dm)
    D[0] = np.maximum(D[0], 1)       # canonical layout covers all nodes
    S = np.zeros((nq, G + 1), dtype=np.int64)
    S[:, 1:] = np.cumsum(D, axis=1)
    F = S[:, -1]                                   # cols per structure
    Gact = np.array([int((D[q] > 0).sum()) for q in range(nq)])

    # --- edge slot assignment ---
    # edge -> (core, structure, row=invq, occurrence within (node,structure))
    eord = np.lexsort((src, dst))                  # group by dst, then src q
    ec = edge_core[eord]
    ep = edge_pos[eord]
    eq = q_of[eord]
    # occurrence counter within (dst, q): edges sorted by (dst, q)
    key_change = np.ones(E, dtype=bool)
    key_change[1:] = (dst[eord][1:] != dst[eord][:-1]) | (eq[1:] != eq[:-1])
    run_id = np.cumsum(key_change) - 1
    run_starts = np.flatnonzero(key_change)
    occ = np.arange(E) - run_starts[run_id]

    row = invq[ec, eq, ep]                         # row index in structure
    g = row // P
    pp = row % P
    col = S[eq, g] + occ
    # slot linear index within (core, structure): i = pp + 128*col
    slot_i = pp + P * col

    # per (core, structure) arrays
    ins = []
    FT = int(F.sum())                              # total cols, all structures
    Scol = np.zeros(nq + 1, dtype=np.int64)
    Scol[1:] = np.cumsum(F)

    # params laid out [P, FT] per core (slot (q,p,col) -> [p, Scol[q]+col])
    par_shape = (n_cores, P, FT)
    thetaA = np.zeros(par_shape, np.float32)
    signA = np.zeros(par_shape, np.float32)
    confA = np.zeros(par_shape, np.float32)
    delayA = np.zeros(par_shape, np.float32)
    maskA = np.zeros(par_shape, np.int8)
    pidx = (ec, pp, Scol[eq] + col)
    thetaA[pidx] = theta[eord]
    signA[pidx] = sign[eord]
    confA[pidx] = conf[eord]
    delayA[pidx] = delay[eord]
    maskA[pidx] = mask8[eord]

    # gather index lists (wrapped) per core: concat over structures of
    # the per-structure slot-order index list (idx relative to q base)
    srcrel = (src[eord] - eq * SRC_CHUNK).astype(np.int16)
    gidx = []
    ncalls = np.zeros(nq, dtype=np.int64)
    for c in range(n_cores):
        parts = []
        for q in range(nq):
            tot = int(F[q]) * P
            a = np.zeros(tot, np.int16)
            selq = (ec == c) & (eq == q)
            a[slot_i[selq]] = srcrel[selq]
            parts.append(_wrap_idx(a, GATHER_CALL))
            ncalls[q] = (tot + GATHER_CALL - 1) // GATHER_CALL
        gidx.append(np.concatenate(parts, axis=1))
    gidx_cols = gidx[0].shape[1]

    # canonical placement = structure 0's; scatter index for structures
    # 1..nq-1: input position i = p + 128*g -> canonical slot p0*G + g0
    sidx = []
    for c in range(n_cores):
        parts = []
        for q in range(1, nq):
            node_pos = ordq[c, q]                  # row j -> position
            crow = invq[c, 0, node_pos]            # canonical row of node
            canon = (crow % P) * G + (crow // P)   # partition-major slot id
            a = np.zeros(nper, np.int16)
            jj = np.arange(nper)
            a[(jj % P) + P * (jj // P)] = canon.astype(np.int16)
            nact = int(Gact[q]) * P
            qtr = (G // 4) * P
            cuts = [0, qtr, 2 * qtr, 3 * qtr, nact]
            for k in range(4):
                ca, cb = min(cuts[k], nact), min(cuts[k + 1], nact)
                if cb > ca:
                    parts.append(_wrap_idx(a[ca:cb], cb - ca))
        sidx.append(np.concatenate(parts, axis=1) if parts else
                    np.zeros((128, 16), np.int16))
    sidx_cols = sidx[0].shape[1]

    # node params in canonical placement [P, G]
    def node_arr(vals, fill):
        a = np.full((n_cores, P, G), fill, dtype=np.float32)
        for c in range(n_cores):
            node_pos = ordq[c, 0]                  # canonical row j -> pos
            rank = node_pos * n_cores + c          # position -> rank
            ok = rank < N
            nd = order[np.minimum(rank, N - 1)]
            v = np.where(ok, vals[nd], fill).astype(np.float32)
            a[c].reshape(-1)[(np.arange(nper) % P) * G +
                             (np.arange(nper) // P)] = np.where(
                                 ok, v, fill)
        return a

    biasA = node_arr(np.asarray(bias), 0.0)
    ratelogA = node_arr(np.asarray(ratelog), 0.0)
    baserateA = node_arr(np.asarray(baserate), 0.0)
    capA = node_arr(np.asarray(cap), 1.0)

    import ml_dtypes
    xdt = ml_dtypes.bfloat16 if XBF16 else np.float32
    xT4 = np.zeros((n_pad, XROW), xdt)
    xT4[:N, :B] = np.asarray(x, dtype=np.float32).T.astype(xdt)

    xTf = np.asarray(x, dtype=np.float32).T
    xownA = np.zeros((n_cores, P, G, B), np.float32)
    node_ids = np.zeros((n_cores, P, G), np.int64)
    for c in range(n_cores):
        node_pos = ordq[c, 0]
        rank = node_pos * n_cores + c
        ok = rank < N
        nd = np.where(ok, order[np.minimum(rank, N - 1)], -1)
        jj = np.arange(nper)
        pcol = (jj % P, jj // P)
        node_ids[c][pcol] = nd
        xownA[c][pcol[0], pcol[1], :] = np.where(
            ok[:, None], xTf[np.maximum(nd, 0), :], 0.0)

    for c in range(n_cores):
        ins.append({
            "xT4": xT4,
            "gidx": gidx[c],
            "sidx": sidx[c],
            "theta": thetaA[c],
            "sgn": signA[c],
            "conf": confA[c],
            "delay": delayA[c],
            "maskf": maskA[c],
            "bias": biasA[c],
            "ratelog": ratelogA[c],
            "baserate": baserateA[c],
            "cap": capA[c],
            "xown": xownA[c].reshape(P, G * B),
        })
    plan = dict(B=B, N=N, G=G, nq=nq, D=D, S=S, F=F, Scol=Scol, Gact=Gact,
                n_pad=n_pad, gidx_cols=gidx_cols, sidx_cols=sidx_cols,
                node_ids=node_ids)
    return ins, plan


def _assemble(results, plan):
    B, N, G = plan["B"], plan["N"], plan["G"]
    out = np.empty((B, N), dtype=np.float32)
    for ci, res in enumerate(results):
        o = res["out"].reshape(P, G, B)
        nid = plan["node_ids"][ci]
        ok = nid >= 0
        out[:, nid[ok]] = o[ok].T
    return out


# ---------------------------------------------------------------------------
# Device kernel
# ---------------------------------------------------------------------------

def _raw_dma_gather(g, out_ap, in_ap, idxs_ap, num_idxs, elem_size, elem_step,
                    queue_num):
    stride_bytes = elem_step * mybir.dt.size(in_ap.dtype)
    return g.add_instruction(
        mybir.InstDMAGatherAnt(
            name=g.bass.get_next_instruction_name(),
            ins=[*g.lower_ap_dma(in_ap, for_custom_bir_dma=True),
                 g.lower_ap(idxs_ap), g.lower_val_access(g.to_reg(num_idxs))],
            outs=[g.lower_ap(out_ap)],
            transpose=False, num_idxs=num_idxs, elem_size=elem_size,
            stride_bytes_256=stride_bytes // 256, gen_mode=0,
            single_packet=False, queue_num=queue_num,
            sbuf_tokens_per_rank=0, sbuf_free_dim_per_rank=0,
            sbuf_free_dim_pad_per_rank=0, sbuf_byte_offset=0))


def _equal_d_runs(D, g0, g1):
    runs = []
    a = g0
    while a < g1:
        b = a + 1
        while b < g1 and D[b] == D[a]:
            b += 1
        runs.append((a, b, int(D[a])))
        a = b
    return runs


def build(B, N, G, nq, D, S, F, Scol, n_pad, gidx_cols, sidx_cols,
          Gact=None, node_ids=None, enable_asserts=False, loop_r=None):
    if Gact is None:
        Gact = np.array([G] * nq)
    FT = int(Scol[-1])
    nc = bacc.Bacc("TRN2", target_bir_lowering=False, debug=False,
                   enable_asserts=enable_asserts, num_swdge_queues=4)

    XDT = mybir.dt.bfloat16 if XBF16 else F32
    xT4 = nc.dram_tensor("xT4", [n_pad, XROW], XDT, kind="ExternalInput")
    giD = nc.dram_tensor("gidx", [128, gidx_cols], I16, kind="ExternalInput")
    siD = nc.dram_tensor("sidx", [128, sidx_cols], I16, kind="ExternalInput")
    thD = nc.dram_tensor("theta", [P, FT], F32, kind="ExternalInput")
    sgD = nc.dram_tensor("sgn", [P, FT], F32, kind="ExternalInput")
    cfD = nc.dram_tensor("conf", [P, FT], F32, kind="ExternalInput")
    dlD = nc.dram_tensor("delay", [P, FT], F32, kind="ExternalInput")
    mkD = nc.dram_tensor("maskf", [P, FT], I8, kind="ExternalInput")
    biD = nc.dram_tensor("bias", [P, G], F32, kind="ExternalInput")
    rlD = nc.dram_tensor("ratelog", [P, G], F32, kind="ExternalInput")
    brD = nc.dram_tensor("baserate", [P, G], F32, kind="ExternalInput")
    cpD = nc.dram_tensor("cap", [P, G], F32, kind="ExternalInput")
    xoD = nc.dram_tensor("xown", [P, G * B], F32, kind="ExternalInput")
    outD = nc.dram_tensor("out", [P, G * B], F32, kind="ExternalOutput")
    # partial-agg merge buffers (zero-initialized by the runtime)
    pagg = [nc.dram_tensor(f"pagg{q}", [G * P, ROWE], F32,
                           kind="ExternalOutput") for q in range(1, nq)]

    Tanh = mybir.ActivationFunctionType.Tanh
    Exp = mybir.ActivationFunctionType.Exp

    qrr = [0]

    def next_q():
        qrr[0] = (qrr[0] + 1) % 4
        return qrr[0]

    import contextlib
    with tile.TileContext(nc) as tc:
        with (
            tc.tile_pool(name="persist", bufs=1) as ppool,
            tc.tile_pool(name="work", bufs=2) as wp,
            tc.tile_pool(name="msgs", bufs=2) as mp,
        ):
          with (tc.For_i(0, loop_r, 1) if loop_r else
                contextlib.nullcontext()):
            agg0 = ppool.tile([P, G * B], F32, tag="agg0")
            aggq_tiles = []

            qorder = list(range(1, nq)) + [0]
            gidx_bases = np.zeros(nq + 1, dtype=np.int64)
            sidx_bases = np.zeros(nq, dtype=np.int64)
            sb = 0
            for q in range(nq):
                tot = int(F[q]) * P
                ncall = (tot + GATHER_CALL - 1) // GATHER_CALL if tot else 0
                gidx_bases[q + 1] = gidx_bases[q] + ncall * (GATHER_CALL // 16)
                if q >= 1:
                    sidx_bases[q] = sb
                    nact = int(Gact[q]) * P
                    sb += (nact + 15) // 16
            # ---- prefetch all idx/param/node streams up front ----
            gidx_t = ppool.tile([128, int(gidx_bases[nq])], I16, tag="gidx")
            nc.sync.dma_start(out=gidx_t[:], in_=giD[:, :])
            pt = {}
            for q in qorder:
                Fq = int(F[q])
                if Fq == 0:
                    continue
                c0, c1 = int(Scol[q]), int(Scol[q + 1])
                th = ppool.tile([P, Fq], F32, tag=f"th{q}")
                sg = ppool.tile([P, Fq], F32, tag=f"sg{q}")
                cf = ppool.tile([P, Fq], F32, tag=f"cf{q}")
                dl = ppool.tile([P, Fq], F32, tag=f"dl{q}")
                mk = ppool.tile([P, Fq], I8, tag=f"mk{q}")
                nc.sync.dma_start(out=th[:], in_=thD[:, c0:c1])
                nc.sync.dma_start(out=sg[:], in_=sgD[:, c0:c1])
                nc.sync.dma_start(out=cf[:], in_=cfD[:, c0:c1])
                nc.sync.dma_start(out=dl[:], in_=dlD[:, c0:c1])
                nc.sync.dma_start(out=mk[:], in_=mkD[:, c0:c1])
                pt[q] = (th, sg, cf, dl, mk)
            bi = ppool.tile([P, G], F32, tag="bi")
            rl = ppool.tile([P, G], F32, tag="rl")
            br = ppool.tile([P, G], F32, tag="br")
            cp = ppool.tile([P, G], F32, tag="cp")
            xo = ppool.tile([P, G * B], F32, tag="xo")
            nc.sync.dma_start(out=bi[:], in_=biD[:, :])
            nc.sync.dma_start(out=rl[:], in_=rlD[:, :])
            nc.sync.dma_start(out=br[:], in_=brD[:, :])
            nc.sync.dma_start(out=cp[:], in_=cpD[:, :])
            nc.sync.dma_start(out=xo[:], in_=xoD[:, :])

            for q in qorder:
                Fq = int(F[q])
                TOTq = Fq * P
                if TOTq == 0:
                    continue
                ncall = (TOTq + GATHER_CALL - 1) // GATHER_CALL
                gb = int(gidx_bases[q])

                (th, sg, cf, dl, mk) = pt[q]
                t = wp.tile([P, Fq], F32, tag="t")
                w = wp.tile([P, Fq], F32, tag="w")
                nc.scalar.activation(t[:], th[:], Tanh)
                nc.vector.tensor_mul(w[:], sg[:], cf[:])
                nc.vector.copy_predicated(w[:], mk[:], t[:])
                nc.vector.tensor_mul(w[:], w[:], dl[:])

                msgs = mp.tile([P, Fq * B], F32, tag="msgs")
                m3 = msgs[:].rearrange("p (s b) -> p s b", b=B)
                base = q * SRC_CHUNK
                in_ap = xT4[base:min(base + SRC_CHUNK, n_pad), :B]
                if XBF16:
                    msgsr = mp.tile([P, Fq * B], XDT, tag="msgsr")
                    gdst = msgsr[:].rearrange("p (s b) -> p s b", b=B)
                else:
                    gdst = m3
                for k in range(ncall):
                    i0 = k * GATHER_CALL
                    ni = min(GATHER_CALL, TOTq - i0)
                    _raw_dma_gather(
                        nc.gpsimd,
                        gdst[:, i0 // P:(i0 + ni) // P, :],
                        in_ap,
                        gidx_t[:, gb + k * (GATHER_CALL // 16):
                               gb + k * (GATHER_CALL // 16) + (ni + 15) // 16],
                        ni, B, XROW, next_q())

                wb = w[:].unsqueeze(-1).to_broadcast([P, Fq, B])
                if XBF16:
                    nc.vector.tensor_tensor(out=m3, in0=gdst, in1=wb,
                                            op=mybir.AluOpType.mult)
                else:
                    nc.vector.tensor_mul(m3, m3, wb)

                if q == 0:
                    aggt = agg0
                else:
                    aggt = wp.tile([P, G * B], F32, tag="aggq")
                    aggq_tiles.append(aggt)
                for (ga, gb2, d) in _equal_d_runs(D[q], 0, int(Gact[q])):
                    if d == 0:
                        continue
                    src_ap = (m3[:, int(S[q, ga]):int(S[q, gb2]), :]
                              .rearrange("p (n d) b -> p n b d", d=d))
                    dst_ap = aggt[:, ga * B:gb2 * B].rearrange(
                        "p (n b) -> p n b", b=B)
                    nc.vector.tensor_reduce(
                        dst_ap, src_ap, axis=mybir.AxisListType.X,
                        op=mybir.AluOpType.add)

                if q > 0:
                    # scatter active rows into canonical order through HBM
                    # (quarters: <=417 ring descs so Pool never blocks)
                    a3 = aggt[:].rearrange("p (g b) -> p g b", b=B)
                    nact = int(Gact[q]) * P
                    qtr = (G // 4) * P
                    cuts = [0, qtr, 2 * qtr, 3 * qtr, nact]
                    sbase = int(sidx_bases[q])
                    off = 0
                    for k in range(4):
                        ca = min(cuts[k], nact)
                        cb = min(cuts[k + 1], nact)
                        if cb <= ca:
                            continue
                        nc.gpsimd.dma_scatter_add(
                            pagg[q - 1][:, :B],
                            a3[:, ca // P:cb // P, :],
                            _slice_idx(wp, nc, siD, sbase + off, cb - ca),
                            cb - ca, cb - ca, B, elem_step=ROWE,
                            single_packet=False, queue_num=next_q())
                        off += (cb - ca) // 16

            # ---- merge + ODE epilogue ----
            rdb = []
            for q in range(1, nq):
                if int(Gact[q]) == 0:
                    continue
                aggt = aggq_tiles[q - 1]
                rt = ppool.tile([P, G * B], F32, tag=f"rdb{q}")
                # WAR on aggt -> waits for the scatter DMAs; RAW+WAW chain
                # orders the pagg readback behind them (DRAM is untracked)
                nc.vector.tensor_scalar_mul(aggt[:, 0:1], aggt[:, 0:1], 1.0)
                nc.vector.tensor_scalar_mul(rt[:, 0:1], aggt[:, 0:1], 1.0)
                nc.sync.dma_start(
                    out=rt[:].rearrange("p (g b) -> p g b", b=B),
                    in_=pagg[q - 1][:, :B].rearrange(
                        "(p g) b -> p g b", p=P))
                rdb.append(rt)
            for rt in rdb:
                nc.vector.tensor_add(agg0[:], agg0[:], rt[:])

            rate = ppool.tile([P, G], F32, tag="rate")
            nc.scalar.activation(rate[:], rl[:], Exp)
            nc.vector.tensor_mul(rate[:], rate[:], br[:])

            a3 = agg0[:].rearrange("p (g b) -> p g b", b=B)
            bib = bi[:].unsqueeze(-1).to_broadcast([P, G, B])
            cpb = cp[:].unsqueeze(-1).to_broadcast([P, G, B])
            rateb = rate[:].unsqueeze(-1).to_broadcast([P, G, B])

            dr = ppool.tile([P, G * B], F32, tag="dr")
            d3 = dr[:].rearrange("p (g b) -> p g b", b=B)
            nc.vector.tensor_add(d3, a3, bib)
            nc.scalar.activation(dr[:], dr[:], Tanh)
            nc.vector.tensor_mul(d3, d3, cpb)
            nc.vector.tensor_tensor(out=dr[:], in0=dr[:], in1=xo[:],
                                    op=mybir.AluOpType.subtract)
            nc.vector.tensor_mul(d3, d3, rateb)
            nc.vector.tensor_scalar_mul(dr[:], dr[:], float(DT))
            nc.vector.tensor_add(dr[:], dr[:], xo[:])
            nc.vector.tensor_scalar_max(dr[:], dr[:], 0.0)
            nc.vector.tensor_tensor(out=d3, in0=d3, in1=cpb,
                                    op=mybir.AluOpType.min)
            nc.sync.dma_start(out=outD[:, :], in_=dr[:])

    nc.compile()
    return nc


def _slice_idx(wp, nc, siD, col0, n):
    t = wp.tile([128, (n + 15) // 16], I16, tag="sidx")
    nc.sync.dma_start(out=t[:], in_=siD[:, col0:col0 + (n + 15) // 16])
    return t[:]


# ---------------------------------------------------------------------------
# Entry point
# ---------------------------------------------------------------------------

def kernel(x, theta_graph, node_bias, rate_log_scale, base_rate, capacity,
           sign_prior, conf_scale, delay_scale, src_index, dst_index,
           learn_mask):
    ins, plan = _prep(x, theta_graph, node_bias, rate_log_scale, base_rate,
                      capacity, sign_prior, conf_scale, delay_scale,
                      src_index, dst_index, learn_mask, NCORES)
    nc = build(plan["B"], plan["N"], plan["G"], plan["nq"], plan["D"],
               plan["S"], plan["F"], plan["Scol"], plan["n_pad"],
               plan["gidx_cols"], plan["sidx_cols"], Gact=plan["Gact"])
    res = run_bass_kernel_spmd(nc, ins, core_ids=list(range(NCORES)))
    return _assemble(res.results, plan)

